# revision 4
# baseline (speedup 1.0000x reference)
"""GTU (gated Toeplitz unit) Bass kernel for 8 TRN2 NeuronCores — v2.

Sharding: tensor-parallel over heads (H=8 -> 1 head/core); host sums the
8 partial o-projections.

v2 vs baseline:
- All big matmuls in bf16 (1 cycle/row on PE vs 4 for fp32); RPE MLP in
  f32r (same storage as fp32, 1 cycle/row at free-dim>=256).
- Kernel lags truncated at L=768 (decay gamma^768 ~ 4.4e-4), shrinking
  the circular conv from 4096 to M2=2816 points.
- One SBUF-resident DFT matrix per phase (loaded once, not per batch);
  forward spectra, complex multiply and gate all stay on-chip; only the
  P spectrum round-trips DRAM between the two DFT phases.
- u/v projections fused into a single pass over x^T.
"""

import numpy as np
import ml_dtypes

B, N, E = 4, 2048, 1024
H = 8
D1 = 3 * E
DH = D1 // H            # 384
R = 512
GAMMA = 0.99
EPS = 1e-8
L = 768                 # truncated kernel lags (6*128)
LC = L // 128           # 6
M2 = 2816               # circular conv length >= N + L - 1
KH = M2 // 2 + 1        # 1409 rfft bins
KC = 12                 # freq chunks of 128 (pad 1409 -> 1536)
KP = KC * 128           # 1536
ROWS = B * N            # 8192
KA = 1152               # augmented contraction for x (bias row), 9*128

_CACHE = {}

bfl = ml_dtypes.bfloat16


def _t3(a, dtype=np.float32):
    """(M, N) -> (128, M/128, N) partition-tiled layout."""
    m, n = a.shape
    assert m % 128 == 0
    return np.ascontiguousarray(
        a.reshape(m // 128, 128, n).transpose(1, 0, 2)).astype(dtype)


def _from3(a):
    p, m, n = a.shape
    return np.ascontiguousarray(
        np.asarray(a, np.float32).transpose(1, 0, 2)).reshape(m * 128, n)


def _consts():
    if "dft" in _CACHE:
        return _CACHE["dft"]
    t = np.arange(N, dtype=np.float64)[:, None]
    k = np.arange(KP, dtype=np.float64)[None, :]
    mask = (k <= (KH - 1)).astype(np.float64)
    ang = 2.0 * np.pi * t * k / M2
    cr = np.cos(ang) * mask
    ci = -np.sin(ang) * mask
    wd = np.concatenate([cr, ci], axis=1)                 # (2048, 3072)

    kk = np.arange(KP, dtype=np.float64)[:, None]
    tt = np.arange(N, dtype=np.float64)[None, :]
    w = np.where((kk == 0) | (kk == M2 // 2), 1.0, 2.0) * (kk <= (KH - 1)) / M2
    ang2 = 2.0 * np.pi * kk * tt / M2
    icos = w * np.cos(ang2)                               # (1536, 2048)
    isin = -w * np.sin(ang2)
    wf = np.concatenate([icos, isin], axis=0)             # (3072, 2048)

    decay = GAMMA ** np.arange(L, dtype=np.float64)       # lag 0 -> 1.0
    decay_t = decay.reshape(LC, 128).T                    # (128, 6)
    _CACHE["dft"] = (_t3(wd, bfl), _t3(wf, bfl), decay_t.astype(np.float32))
    return _CACHE["dft"]


def _build():
    import concourse.bass as bass
    import concourse.mybir as mybir
    import concourse.tile as tile
    from concourse import bacc

    AFT = mybir.ActivationFunctionType
    ALU = mybir.AluOpType
    f32 = mybir.dt.float32
    f32r = mybir.dt.float32r
    bf16 = mybir.dt.bfloat16

    nc = bacc.Bacc(None, target_bir_lowering=False, debug=False, num_devices=8)

    def din(name, shape, dt=f32):
        return nc.dram_tensor(name, list(shape), dt, kind="ExternalInput")

    def dint(name, shape, dt=bf16):
        return nc.dram_tensor(name, list(shape), dt)

    # inputs
    xTa = din("xTa", (128, KA // 128, ROWS), bf16)
    u_wa = din("u_wa", (128, KA // 128, DH), bf16)
    v_wa = din("v_wa", (128, KA // 128, DH), bf16)
    o_w3 = din("o_w3", (128, DH // 128, E), bf16)
    wd_d = din("wd", (128, N // 128, 2 * KP), bf16)
    wf_d = din("wf", (128, 2 * KC, N), bf16)
    p_aug = din("p_aug", (2, L))
    pw_aug = din("pw_aug", (2, R))
    lws = [din(f"lw{i}", (128, R // 128, R), bf16) for i in range(3)]
    lbs = din("lbs", (128, 3 * (R // 128)))
    out_w3 = din("out_w3", (128, R // 128, DH), bf16)
    outb = din("outb", (1, DH))
    decay = din("decay", (128, LC))
    out = nc.dram_tensor("out", [128, ROWS // 128, E], f32,
                         kind="ExternalOutput")

    # dram temps (bf16)
    uT_d = dint("uT_d", (128, DH // 128, ROWS))
    v_d = dint("v_d", (128, ROWS // 128, DH))
    psp_d = dint("psp_d", (128, B * 2 * KC, DH))

    FG = R // 128             # 4 feature groups
    NCH = L // 384            # 2 position chunks in the (truncated) MLP

    with tile.TileContext(nc) as tc, nc.allow_low_precision(
            reason="bf16 pipeline validated against fp32 reference"):
        with tc.tile_pool(name="persist", bufs=1) as pp:
            acoef = pp.tile([128, LC, DH], bf16)   # truncated decayed coefs

            # wd is loaded up front: its pool sits above the phase-0/1
            # working set, so the 12.6MB DMA overlaps the MLP + u/v phase
            # instead of stalling the DFT phase behind it.
            wd_pool = tc.tile_pool(name="wd", bufs=1)
            wdp = wd_pool.__enter__()
            wd_sb = wdp.tile([128, N // 128, 2 * KP], bf16)
            for kc in range(N // 128):
                nc.scalar.dma_start(wd_sb[:, kc, :], wd_d[:, kc, :])

            # ------- RPE MLP + u/v projections (concurrent engines) -----
            # The MLP's serial norm->matmul chains leave the PE idle; the
            # u/v GEMMs stream through the same window and fill it.
            with (tc.tile_pool(name="mlp", bufs=1) as mp,
                  tc.tile_pool(name="mlp_ps", bufs=1, space="PSUM") as mps,
                  tc.tile_pool(name="uvw", bufs=1) as wp,
                  tc.tile_pool(name="uvx", bufs=3) as xp,
                  tc.tile_pool(name="uvs", bufs=4) as sp,
                  tc.tile_pool(name="uv_ps", bufs=2, space="PSUM") as ups):
                uw_sb = wp.tile([128, KA // 128, DH], bf16)
                vw_sb = wp.tile([128, KA // 128, DH], bf16)
                nc.sync.dma_start(uw_sb[:], u_wa[:])
                nc.sync.dma_start(vw_sb[:], v_wa[:])
                for grp in range(ROWS // 512):
                    xt = xp.tile([128, KA // 128, 512], bf16, name="xt",
                                 tag="xt")
                    nc.sync.dma_start(
                        xt[:], xTa[:, :, grp * 512:(grp + 1) * 512])
                    # uT tile: out[M=DH, N=512 rows]
                    for m in range(DH // 128):
                        ps = ups.tile([128, 512], f32, name="bps", tag="bps")
                        for kc in range(KA // 128):
                            nc.tensor.matmul(
                                ps[:], uw_sb[:, kc, m * 128:(m + 1) * 128],
                                xt[:, kc, :], start=(kc == 0),
                                stop=(kc == KA // 128 - 1))
                        ut = sp.tile([128, 512], bf16, name="ut", tag="ut")
                        nc.scalar.activation(ut[:], ps[:], AFT.Silu)
                        nc.sync.dma_start(
                            uT_d[:, m, grp * 512:(grp + 1) * 512], ut[:])
                    # v tiles: out[M=128 rows, N=DH]
                    for rs in range(4):
                        ps = ups.tile([128, DH], f32, name="cps", tag="cps")
                        for kc in range(KA // 128):
                            nc.tensor.matmul(
                                ps[:], xt[:, kc, rs * 128:(rs + 1) * 128],
                                vw_sb[:, kc, :], start=(kc == 0),
                                stop=(kc == KA // 128 - 1))
                        vt = sp.tile([128, DH], bf16, name="vt", tag="vt")
                        nc.scalar.activation(vt[:], ps[:], AFT.Silu)
                        nc.sync.dma_start(v_d[:, grp * 4 + rs, :], vt[:])
                ones_col = mp.tile([128, 1], bf16)     # K=128 -> M=1 reducer
                nc.vector.memset(ones_col[:], 1.0)
                one_row = mp.tile([1, 128], bf16)      # K=1 -> 128-part bcast
                nc.vector.memset(one_row[:], 1.0)
                one_rowf = mp.tile([1, 128], f32)
                nc.vector.memset(one_rowf[:], 1.0)
                c_sc = mp.tile([1, 1], f32)
                nc.vector.memset(c_sc[:], float(R ** -0.5))
                eps_sc = mp.tile([1, 1], f32)
                nc.vector.memset(eps_sc[:], EPS)

                pa_sb = mp.tile([2, L], f32)
                pw_sb = mp.tile([2, R], f32)
                lb_sb = mp.tile([128, 3 * FG], f32)
                nc.sync.dma_start(pa_sb[:], p_aug[:])
                nc.sync.dma_start(pw_sb[:], pw_aug[:])
                nc.sync.dma_start(lb_sb[:], lbs[:])

                # MLP runs only on the L kept lags; fp32 h, bf16 matmuls
                h = [mp.tile([128, L], f32, name=f"h{g}", tag=f"h{g}")
                     for g in range(FG)]
                # h0 = pos_idx @ pos_w + pos_b (K=2, fp32), feature-major
                for g in range(FG):
                    for nch in range(NCH):
                        ps = mps.tile([128, 384], f32, name="mmps", tag="mm")
                        nc.tensor.matmul(
                            ps[:], pw_sb[:, g * 128:(g + 1) * 128],
                            pa_sb[:, nch * 384:(nch + 1) * 384],
                            start=True, stop=True)
                        nc.vector.tensor_copy(
                            h[g][:, nch * 384:(nch + 1) * 384], ps[:])

                def srms_relu(h_in, phi_out):
                    # s[t] = sum_f h^2 ; factor = 1/(sqrt(s)/sqrt(R) + eps)
                    sq = [mp.tile([128, L], bf16, name=f"sq{g}", tag=f"sq{g}")
                          for g in range(FG)]
                    for g in range(FG):
                        nc.vector.tensor_mul(sq[g][:], h_in[g][:], h_in[g][:])
                    facb = mp.tile([1, L], bf16, name="facb", tag="facb")
                    fac = mp.tile([1, L], f32, name="fac", tag="fac")
                    for nch in range(NCH):
                        ps1 = mps.tile([1, 384], f32, name="redps", tag="red")
                        for g in range(FG):
                            nc.tensor.matmul(
                                ps1[:], ones_col[:],
                                sq[g][:, nch * 384:(nch + 1) * 384],
                                start=(g == 0), stop=(g == FG - 1))
                        sl = fac[:, nch * 384:(nch + 1) * 384]
                        nc.scalar.activation(sl, ps1[:], AFT.Sqrt)
                        nc.vector.tensor_scalar(
                            sl, sl, c_sc[:], eps_sc[:], ALU.mult, ALU.add)
                        nc.vector.reciprocal(
                            facb[:, nch * 384:(nch + 1) * 384], sl)
                    fb = mp.tile([128, L], f32, name="fb", tag="fb")
                    for nch in range(NCH):
                        psb = mps.tile([128, 384], f32, name="bcps", tag="bc")
                        nc.tensor.matmul(
                            psb[:], one_row[:],
                            facb[:, nch * 384:(nch + 1) * 384],
                            start=True, stop=True)
                        nc.vector.tensor_copy(
                            fb[:, nch * 384:(nch + 1) * 384], psb[:])
                    for g in range(FG):
                        nc.vector.tensor_mul(phi_out[g][:], h_in[g][:], fb[:])
                        nc.scalar.activation(
                            phi_out[g][:], phi_out[g][:], AFT.Relu)

                phi = [mp.tile([128, L], bf16, name=f"phi{g}", tag=f"phi{g}")
                       for g in range(FG)]
                srms_relu(h, phi)

                lw_sb = mp.tile([128, FG, R], bf16)
                for li in range(3):
                    nc.sync.dma_start(lw_sb[:], lws[li][:])
                    for g in range(FG):
                        for nch in range(NCH):
                            ps = mps.tile([128, 384], f32, name="mmps",
                                          tag="mm")
                            for kk in range(FG):
                                nc.tensor.matmul(
                                    ps[:],
                                    lw_sb[:, kk, g * 128:(g + 1) * 128],
                                    phi[kk][:, nch * 384:(nch + 1) * 384],
                                    start=(kk == 0), stop=(kk == FG - 1))
                            sl = h[g][:, nch * 384:(nch + 1) * 384]
                            nc.vector.tensor_scalar(
                                sl, ps[:],
                                lb_sb[:, li * FG + g:li * FG + g + 1],
                                None, ALU.add)
                    srms_relu(h, phi)

                # coefs (t-major, lags < L only) = phi.T @ out_w
                ow_sb = mp.tile([128, FG, DH], bf16)
                ob_sb = mp.tile([1, DH], f32)
                dec_sb = mp.tile([128, LC], f32)
                nc.sync.dma_start(ow_sb[:], out_w3[:])
                nc.sync.dma_start(ob_sb[:], outb[:])
                nc.sync.dma_start(dec_sb[:], decay[:])
                obb = mp.tile([128, DH], f32)
                psb = mps.tile([128, DH], f32, name="bc2ps", tag="bc")
                nc.tensor.matmul(psb[:], one_rowf[:], ob_sb[:],
                                 start=True, stop=True)
                nc.vector.tensor_copy(obb[:], psb[:])
                for m in range(LC):
                    ps = mps.tile([128, DH], f32, name="mm2ps", tag="mm")
                    for kk in range(FG):
                        nc.tensor.matmul(
                            ps[:], phi[kk][:, m * 128:(m + 1) * 128],
                            ow_sb[:, kk, :], start=(kk == 0),
                            stop=(kk == FG - 1))
                    ac = mp.tile([128, DH], f32, name="ac", tag="ac")
                    nc.vector.tensor_add(ac[:], ps[:], obb[:])
                    nc.vector.tensor_scalar(
                        acoef[:, m, :], ac[:], dec_sb[:, m:m + 1],
                        None, ALU.mult)

            # ---------------- forward DFTs + complex multiply -----------
            # m-tile KC+11 (sin rows at the Nyquist chunk) is identically
            # zero: sin(pi*t) = 0. Skip it in A/D and its product in F;
            # at j=11 only the real product survives.
            with (tc.tile_pool(name="fwd", bufs=1) as fp_,
                  tc.tile_pool(name="fwdx", bufs=2) as fpx,
                  tc.tile_pool(name="fwdv", bufs=2) as fpv,
                  tc.tile_pool(name="fwd2", bufs=4) as fp2,
                  tc.tile_pool(name="fwd_ps", bufs=4, space="PSUM") as fps):
                asp = fp_.tile([128, 2 * KC, DH], bf16)    # kernel spectrum
                # A: Ar/Ai m-tiles, contraction over L lags only
                for m in range(2 * KC - 1):
                    ps = fps.tile([128, DH], f32, name="aps", tag="aps")
                    for kc in range(LC):
                        nc.tensor.matmul(
                            ps[:], wd_sb[:, kc, m * 128:(m + 1) * 128],
                            acoef[:, kc, :], start=(kc == 0),
                            stop=(kc == LC - 1))
                    nc.scalar.activation(asp[:, m, :], ps[:], AFT.Copy)
                for b in range(B):
                    vb = fpv.tile([128, N // 128, DH], bf16, name="vb",
                                  tag="vb")
                    nc.sync.dma_start(
                        vb[:], v_d[:, b * (N // 128):(b + 1) * (N // 128), :])
                    xsp = fpx.tile([128, 2 * KC, DH], bf16, name="xsp",
                                   tag="xsp")
                    # D: X = DFT(v_b); interleave re/im pairs for E
                    for j in range(KC):
                        ms = (j,) if j == KC - 1 else (j, KC + j)
                        for m in ms:
                            ps = fps.tile([128, DH], f32, name="dps",
                                          tag="dps")
                            for kc in range(N // 128):
                                nc.tensor.matmul(
                                    ps[:],
                                    wd_sb[:, kc, m * 128:(m + 1) * 128],
                                    vb[:, kc, :], start=(kc == 0),
                                    stop=(kc == N // 128 - 1))
                            nc.scalar.activation(xsp[:, m, :], ps[:],
                                                 AFT.Copy)
                        # E: P = A * X (complex), in place over xsp
                        xr, xi = xsp[:, j, :], xsp[:, KC + j, :]
                        ar, ai = asp[:, j, :], asp[:, KC + j, :]
                        if j == KC - 1:
                            nc.vector.tensor_mul(xr, ar, xr)
                            nc.sync.dma_start(
                                psp_d[:, b * 2 * KC + j, :], xr)
                            continue
                        t1 = fp2.tile([128, DH], bf16, name="t1", tag="t1")
                        t2 = fp2.tile([128, DH], bf16, name="t2", tag="t2")
                        t3 = fp2.tile([128, DH], bf16, name="t3", tag="t3")
                        nc.vector.tensor_mul(t1[:], ar, xr)
                        nc.vector.tensor_mul(t2[:], ai, xi)
                        nc.vector.tensor_mul(t3[:], ar, xi)
                        nc.vector.tensor_mul(xi, ai, xr)
                        nc.vector.tensor_sub(xr, t1[:], t2[:])
                        nc.vector.tensor_add(xi, t3[:], xi)
                        nc.sync.dma_start(psp_d[:, b * 2 * KC + j, :], xr)
                        nc.sync.dma_start(
                            psp_d[:, b * 2 * KC + KC + j, :], xi)
            wd_pool.__exit__(None, None, None)

            # ---------------- inverse DFT + gate + o-projection ---------
            with (tc.tile_pool(name="wf", bufs=1) as wfp,
                  tc.tile_pool(name="inv", bufs=2) as ip_,
                  tc.tile_pool(name="invg", bufs=2) as gp_,
                  tc.tile_pool(name="invw", bufs=1) as owp,
                  tc.tile_pool(name="invs", bufs=4) as isp,
                  tc.tile_pool(name="inv_ps", bufs=4, space="PSUM") as ips):
                NJ = 2 * KC - 1        # Nyquist-sin chunk dropped
                wf_sb = wfp.tile([128, NJ, N], bf16)
                for j in range(NJ):    # per-chunk: F pipelines with the load
                    nc.scalar.dma_start(wf_sb[:, j, :], wf_d[:, j, :])
                ow_sb = owp.tile([128, DH // 128, E], bf16)
                nc.sync.dma_start(ow_sb[:], o_w3[:])
                for b in range(B):
                    pb = ip_.tile([128, NJ, DH], bf16, name="pb", tag="pb")
                    ub = ip_.tile([128, DH // 128, N], bf16, name="ub",
                                  tag="ub")
                    nc.sync.dma_start(
                        pb[:], psp_d[:, b * 2 * KC:b * 2 * KC + NJ, :])
                    nc.sync.dma_start(
                        ub[:], uT_d[:, :, b * N:(b + 1) * N])
                    gb = gp_.tile([128, DH // 128, N], bf16, name="gb",
                                  tag="gb")
                    # F: tv^T = sum_k P * WF ; gate with u in the evict
                    for m in range(DH // 128):
                        for tch in range(N // 512):
                            ps = ips.tile([128, 512], f32, name="fps",
                                          tag="fps")
                            for j in range(NJ):
                                nc.tensor.matmul(
                                    ps[:], pb[:, j, m * 128:(m + 1) * 128],
                                    wf_sb[:, j, tch * 512:(tch + 1) * 512],
                                    start=(j == 0), stop=(j == NJ - 1))
                            sl = slice(tch * 512, (tch + 1) * 512)
                            nc.vector.tensor_mul(
                                gb[:, m, sl], ps[:], ub[:, m, sl])
                    # H: partial o-projection out_b = g^T @ o_w
                    for mo in range(N // 128):
                        for ech in range(E // 512):
                            ps = ips.tile([128, 512], f32, name="hps",
                                          tag="hps")
                            for kc in range(DH // 128):
                                nc.tensor.matmul(
                                    ps[:], gb[:, kc, mo * 128:(mo + 1) * 128],
                                    ow_sb[:, kc, ech * 512:(ech + 1) * 512],
                                    start=(kc == 0), stop=(kc == DH // 128 - 1))
                            ot = isp.tile([128, 512], f32, name="ot",
                                          tag="ot")
                            nc.scalar.activation(ot[:], ps[:], AFT.Copy)
                            nc.sync.dma_start(
                                out[:, b * (N // 128) + mo,
                                    ech * 512:(ech + 1) * 512], ot[:])

    nc.compile()
    return nc


def _get_nc():
    if "nc" not in _CACHE:
        _CACHE["nc"] = _build()
    return _CACHE["nc"]


def kernel(x, u_w, u_b, v_w, v_b, o_w, o_b,
           pos_w, pos_b, lw0, lb0, lw1, lb1, lw2, lb2, out_w, out_b):
    from concourse.bass_utils import run_bass_kernel_spmd

    wd3, wf3, decay_t = _consts()
    x_flat = np.asarray(x, np.float32).reshape(ROWS, E)
    xTa = np.zeros((KA, ROWS), np.float32)
    xTa[:E] = x_flat.T
    xTa[E] = 1.0
    xTa3 = _t3(xTa, bfl)

    p_aug = np.stack([np.arange(L, dtype=np.float32),
                      np.ones(L, np.float32)])
    pw_aug = np.concatenate([pos_w, pos_b[None, :]], 0).astype(np.float32)
    lbs = np.concatenate(
        [bb.reshape(R // 128, 128).T for bb in (lb0, lb1, lb2)],
        axis=1).astype(np.float32)

    in_maps = []
    for h in range(H):
        sl = slice(h * DH, (h + 1) * DH)
        u_wa = np.zeros((KA, DH), np.float32)
        u_wa[:E] = u_w[:, sl]
        u_wa[E] = u_b[sl]
        v_wa = np.zeros((KA, DH), np.float32)
        v_wa[:E] = v_w[:, sl]
        v_wa[E] = v_b[sl]
        in_maps.append(dict(
            xTa=xTa3, u_wa=_t3(u_wa, bfl), v_wa=_t3(v_wa, bfl),
            o_w3=_t3(np.ascontiguousarray(o_w[sl, :]).astype(np.float32), bfl),
            wd=wd3, wf=wf3,
            p_aug=p_aug, pw_aug=pw_aug,
            lw0=_t3(lw0, bfl), lw1=_t3(lw1, bfl), lw2=_t3(lw2, bfl), lbs=lbs,
            out_w3=_t3(np.ascontiguousarray(out_w[:, sl]), bfl),
            outb=np.ascontiguousarray(out_b[None, sl]).astype(np.float32),
            decay=decay_t,
        ))

    nc = _get_nc()
    res = run_bass_kernel_spmd(nc, in_maps, core_ids=list(range(8)),
                               trace=bool(_CACHE.get("trace")))
    _CACHE["last_res"] = res
    acc = np.zeros((ROWS, E), np.float32)
    for i in range(H):
        acc += _from3(res.results[i]["out"])
    acc += o_b[None, :]
    return acc.reshape(B, N, E)


# revision 5
# speedup vs baseline: 22472.8311x; 22472.8311x over previous
"""GTU (gated Toeplitz unit) Bass kernel for 8 TRN2 NeuronCores — v2.

Sharding: tensor-parallel over heads (H=8 -> 1 head/core); host sums the
8 partial o-projections.

vs the fp32 baseline (4.98 ms -> 0.94 ms simulated):
- All matmuls in bf16 (1 PE cycle/row vs 4 for fp32); fp32 PSUM
  accumulation; norm math in the RPE MLP stays fp32.
- Kernel lags truncated at L=768 (decay gamma^768 ~ 4.4e-4), shrinking
  the circular conv from 4096 to M2=2816 points and the RPE MLP to the
  768 positions that survive the decay.
- One SBUF-resident DFT matrix per phase, loaded once (not per batch)
  and prefetched on the second (Activation) DMA queue; forward spectra,
  complex multiply and gate all stay on-chip; only the P spectrum
  round-trips DRAM between the two DFT phases. The all-zero
  Nyquist-sine chunk is skipped everywhere.
- u/v projections fused into one pass streaming x^T tiles used as both
  moving (uT) and stationary (v) matmul operands, overlapped with the
  serial MLP chains to keep the PE fed.
"""

import numpy as np
import ml_dtypes

B, N, E = 4, 2048, 1024
H = 8
D1 = 3 * E
DH = D1 // H            # 384
R = 512
GAMMA = 0.99
EPS = 1e-8
L = 768                 # truncated kernel lags (6*128)
LC = L // 128           # 6
M2 = 2816               # circular conv length >= N + L - 1
KH = M2 // 2 + 1        # 1409 rfft bins
KC = 12                 # freq chunks of 128 (pad 1409 -> 1536)
KP = KC * 128           # 1536
ROWS = B * N            # 8192
KA = 1152               # augmented contraction for x (bias row), 9*128

_CACHE = {}

bfl = ml_dtypes.bfloat16


def _t3(a, dtype=np.float32):
    """(M, N) -> (128, M/128, N) partition-tiled layout."""
    m, n = a.shape
    assert m % 128 == 0
    return np.ascontiguousarray(
        a.reshape(m // 128, 128, n).transpose(1, 0, 2)).astype(dtype)


def _from3(a):
    p, m, n = a.shape
    return np.ascontiguousarray(
        np.asarray(a, np.float32).transpose(1, 0, 2)).reshape(m * 128, n)


def _consts():
    if "dft" in _CACHE:
        return _CACHE["dft"]
    t = np.arange(N, dtype=np.float64)[:, None]
    k = np.arange(KP, dtype=np.float64)[None, :]
    mask = (k <= (KH - 1)).astype(np.float64)
    ang = 2.0 * np.pi * t * k / M2
    cr = np.cos(ang) * mask
    ci = -np.sin(ang) * mask
    wd = np.concatenate([cr, ci], axis=1)                 # (2048, 3072)

    kk = np.arange(KP, dtype=np.float64)[:, None]
    tt = np.arange(N, dtype=np.float64)[None, :]
    w = np.where((kk == 0) | (kk == M2 // 2), 1.0, 2.0) * (kk <= (KH - 1)) / M2
    ang2 = 2.0 * np.pi * kk * tt / M2
    icos = w * np.cos(ang2)                               # (1536, 2048)
    isin = -w * np.sin(ang2)
    wf = np.concatenate([icos, isin], axis=0)             # (3072, 2048)

    decay = GAMMA ** np.arange(L, dtype=np.float64)       # lag 0 -> 1.0
    decay_t = decay.reshape(LC, 128).T                    # (128, 6)
    _CACHE["dft"] = (_t3(wd, bfl), _t3(wf, bfl), decay_t.astype(np.float32))
    return _CACHE["dft"]


def _build():
    import concourse.bass as bass
    import concourse.mybir as mybir
    import concourse.tile as tile
    from concourse import bacc

    AFT = mybir.ActivationFunctionType
    ALU = mybir.AluOpType
    f32 = mybir.dt.float32
    f32r = mybir.dt.float32r
    bf16 = mybir.dt.bfloat16

    nc = bacc.Bacc(None, target_bir_lowering=False, debug=False, num_devices=8)

    def din(name, shape, dt=f32):
        return nc.dram_tensor(name, list(shape), dt, kind="ExternalInput")

    def dint(name, shape, dt=bf16):
        return nc.dram_tensor(name, list(shape), dt)

    # inputs
    xTa = din("xTa", (128, KA // 128, ROWS), bf16)
    u_wa = din("u_wa", (128, KA // 128, DH), bf16)
    v_wa = din("v_wa", (128, KA // 128, DH), bf16)
    o_w3 = din("o_w3", (128, DH // 128, E), bf16)
    wd_d = din("wd", (128, N // 128, 2 * KP), bf16)
    wf_d = din("wf", (128, 2 * KC, N), bf16)
    p_aug = din("p_aug", (2, L))
    pw_aug = din("pw_aug", (2, R))
    lws = [din(f"lw{i}", (128, R // 128, R), bf16) for i in range(3)]
    lbs = din("lbs", (128, 3 * (R // 128)))
    out_w3 = din("out_w3", (128, R // 128, DH), bf16)
    outb = din("outb", (1, DH))
    decay = din("decay", (128, LC))
    out = nc.dram_tensor("out", [128, ROWS // 128, E], f32,
                         kind="ExternalOutput")

    # dram temps (bf16)
    uT_d = dint("uT_d", (128, DH // 128, ROWS))
    v_d = dint("v_d", (128, ROWS // 128, DH))
    psp_d = dint("psp_d", (128, B * 2 * KC, DH))

    FG = R // 128             # 4 feature groups
    NCH = L // 384            # 2 position chunks in the (truncated) MLP

    with tile.TileContext(nc) as tc, nc.allow_low_precision(
            reason="bf16 pipeline validated against fp32 reference"):
        with tc.tile_pool(name="persist", bufs=1) as pp:
            acoef = pp.tile([128, LC, DH], bf16)   # truncated decayed coefs

            # wd is loaded up front: its pool sits above the phase-0/1
            # working set, so the 12.6MB DMA overlaps the MLP + u/v phase
            # instead of stalling the DFT phase behind it.
            wd_pool = tc.tile_pool(name="wd", bufs=1)
            wdp = wd_pool.__enter__()
            wd_sb = wdp.tile([128, N // 128, 2 * KP], bf16)
            for kc in range(N // 128):
                nc.scalar.dma_start(wd_sb[:, kc, :], wd_d[:, kc, :])

            # ------- RPE MLP + u/v projections (concurrent engines) -----
            # The MLP's serial norm->matmul chains leave the PE idle; the
            # u/v GEMMs stream through the same window and fill it.
            with (tc.tile_pool(name="mlp", bufs=1) as mp,
                  tc.tile_pool(name="mlp_ps", bufs=1, space="PSUM") as mps,
                  tc.tile_pool(name="uvw", bufs=1) as wp,
                  tc.tile_pool(name="uvx", bufs=3) as xp,
                  tc.tile_pool(name="uvs", bufs=4) as sp,
                  tc.tile_pool(name="uv_ps", bufs=2, space="PSUM") as ups):
                uw_sb = wp.tile([128, KA // 128, DH], bf16)
                vw_sb = wp.tile([128, KA // 128, DH], bf16)
                nc.sync.dma_start(uw_sb[:], u_wa[:])
                nc.sync.dma_start(vw_sb[:], v_wa[:])
                for grp in range(ROWS // 512):
                    xt = xp.tile([128, KA // 128, 512], bf16, name="xt",
                                 tag="xt")
                    nc.sync.dma_start(
                        xt[:], xTa[:, :, grp * 512:(grp + 1) * 512])
                    # uT tile: out[M=DH, N=512 rows]
                    for m in range(DH // 128):
                        ps = ups.tile([128, 512], f32, name="bps", tag="bps")
                        for kc in range(KA // 128):
                            nc.tensor.matmul(
                                ps[:], uw_sb[:, kc, m * 128:(m + 1) * 128],
                                xt[:, kc, :], start=(kc == 0),
                                stop=(kc == KA // 128 - 1))
                        ut = sp.tile([128, 512], bf16, name="ut", tag="ut")
                        nc.scalar.activation(ut[:], ps[:], AFT.Silu)
                        nc.sync.dma_start(
                            uT_d[:, m, grp * 512:(grp + 1) * 512], ut[:])
                    # v tiles: out[M=128 rows, N=DH]
                    for rs in range(4):
                        ps = ups.tile([128, DH], f32, name="cps", tag="cps")
                        for kc in range(KA // 128):
                            nc.tensor.matmul(
                                ps[:], xt[:, kc, rs * 128:(rs + 1) * 128],
                                vw_sb[:, kc, :], start=(kc == 0),
                                stop=(kc == KA // 128 - 1))
                        vt = sp.tile([128, DH], bf16, name="vt", tag="vt")
                        nc.scalar.activation(vt[:], ps[:], AFT.Silu)
                        nc.sync.dma_start(v_d[:, grp * 4 + rs, :], vt[:])
                ones_col = mp.tile([128, 1], bf16)     # K=128 -> M=1 reducer
                nc.vector.memset(ones_col[:], 1.0)
                one_row = mp.tile([1, 128], bf16)      # K=1 -> 128-part bcast
                nc.vector.memset(one_row[:], 1.0)
                one_rowf = mp.tile([1, 128], f32)
                nc.vector.memset(one_rowf[:], 1.0)
                c_sc = mp.tile([1, 1], f32)
                nc.vector.memset(c_sc[:], float(R ** -0.5))
                eps_sc = mp.tile([1, 1], f32)
                nc.vector.memset(eps_sc[:], EPS)

                pa_sb = mp.tile([2, L], f32)
                pw_sb = mp.tile([2, R], f32)
                lb_sb = mp.tile([128, 3 * FG], f32)
                nc.sync.dma_start(pa_sb[:], p_aug[:])
                nc.sync.dma_start(pw_sb[:], pw_aug[:])
                nc.sync.dma_start(lb_sb[:], lbs[:])

                # MLP runs only on the L kept lags; fp32 h, bf16 matmuls
                h = [mp.tile([128, L], f32, name=f"h{g}", tag=f"h{g}")
                     for g in range(FG)]
                # h0 = pos_idx @ pos_w + pos_b (K=2, fp32), feature-major
                for g in range(FG):
                    for nch in range(NCH):
                        ps = mps.tile([128, 384], f32, name="mmps", tag="mm")
                        nc.tensor.matmul(
                            ps[:], pw_sb[:, g * 128:(g + 1) * 128],
                            pa_sb[:, nch * 384:(nch + 1) * 384],
                            start=True, stop=True)
                        nc.vector.tensor_copy(
                            h[g][:, nch * 384:(nch + 1) * 384], ps[:])

                def srms_relu(h_in, phi_out):
                    # s[t] = sum_f h^2 ; factor = 1/(sqrt(s)/sqrt(R) + eps)
                    sq = [mp.tile([128, L], bf16, name=f"sq{g}", tag=f"sq{g}")
                          for g in range(FG)]
                    for g in range(FG):
                        nc.vector.tensor_mul(sq[g][:], h_in[g][:], h_in[g][:])
                    facb = mp.tile([1, L], bf16, name="facb", tag="facb")
                    fac = mp.tile([1, L], f32, name="fac", tag="fac")
                    for nch in range(NCH):
                        ps1 = mps.tile([1, 384], f32, name="redps", tag="red")
                        for g in range(FG):
                            nc.tensor.matmul(
                                ps1[:], ones_col[:],
                                sq[g][:, nch * 384:(nch + 1) * 384],
                                start=(g == 0), stop=(g == FG - 1))
                        sl = fac[:, nch * 384:(nch + 1) * 384]
                        nc.scalar.activation(sl, ps1[:], AFT.Sqrt)
                        nc.vector.tensor_scalar(
                            sl, sl, c_sc[:], eps_sc[:], ALU.mult, ALU.add)
                        nc.vector.reciprocal(
                            facb[:, nch * 384:(nch + 1) * 384], sl)
                    fb = mp.tile([128, L], f32, name="fb", tag="fb")
                    for nch in range(NCH):
                        psb = mps.tile([128, 384], f32, name="bcps", tag="bc")
                        nc.tensor.matmul(
                            psb[:], one_row[:],
                            facb[:, nch * 384:(nch + 1) * 384],
                            start=True, stop=True)
                        nc.vector.tensor_copy(
                            fb[:, nch * 384:(nch + 1) * 384], psb[:])
                    for g in range(FG):
                        nc.vector.tensor_mul(phi_out[g][:], h_in[g][:], fb[:])
                        nc.scalar.activation(
                            phi_out[g][:], phi_out[g][:], AFT.Relu)

                phi = [mp.tile([128, L], bf16, name=f"phi{g}", tag=f"phi{g}")
                       for g in range(FG)]
                srms_relu(h, phi)

                lw_sb = mp.tile([128, FG, R], bf16)
                for li in range(3):
                    nc.sync.dma_start(lw_sb[:], lws[li][:])
                    for g in range(FG):
                        for nch in range(NCH):
                            ps = mps.tile([128, 384], f32, name="mmps",
                                          tag="mm")
                            for kk in range(FG):
                                nc.tensor.matmul(
                                    ps[:],
                                    lw_sb[:, kk, g * 128:(g + 1) * 128],
                                    phi[kk][:, nch * 384:(nch + 1) * 384],
                                    start=(kk == 0), stop=(kk == FG - 1))
                            sl = h[g][:, nch * 384:(nch + 1) * 384]
                            nc.vector.tensor_scalar(
                                sl, ps[:],
                                lb_sb[:, li * FG + g:li * FG + g + 1],
                                None, ALU.add)
                    srms_relu(h, phi)

                # coefs (t-major, lags < L only) = phi.T @ out_w
                ow_sb = mp.tile([128, FG, DH], bf16)
                ob_sb = mp.tile([1, DH], f32)
                dec_sb = mp.tile([128, LC], f32)
                nc.sync.dma_start(ow_sb[:], out_w3[:])
                nc.sync.dma_start(ob_sb[:], outb[:])
                nc.sync.dma_start(dec_sb[:], decay[:])
                obb = mp.tile([128, DH], f32)
                psb = mps.tile([128, DH], f32, name="bc2ps", tag="bc")
                nc.tensor.matmul(psb[:], one_rowf[:], ob_sb[:],
                                 start=True, stop=True)
                nc.vector.tensor_copy(obb[:], psb[:])
                for m in range(LC):
                    ps = mps.tile([128, DH], f32, name="mm2ps", tag="mm")
                    for kk in range(FG):
                        nc.tensor.matmul(
                            ps[:], phi[kk][:, m * 128:(m + 1) * 128],
                            ow_sb[:, kk, :], start=(kk == 0),
                            stop=(kk == FG - 1))
                    ac = mp.tile([128, DH], f32, name="ac", tag="ac")
                    nc.vector.tensor_add(ac[:], ps[:], obb[:])
                    nc.vector.tensor_scalar(
                        acoef[:, m, :], ac[:], dec_sb[:, m:m + 1],
                        None, ALU.mult)

            # ---------------- forward DFTs + complex multiply -----------
            # m-tile KC+11 (sin rows at the Nyquist chunk) is identically
            # zero: sin(pi*t) = 0. Skip it in A/D and its product in F;
            # at j=11 only the real product survives.
            with (tc.tile_pool(name="fwd", bufs=1) as fp_,
                  tc.tile_pool(name="fwdx", bufs=2) as fpx,
                  tc.tile_pool(name="fwdv", bufs=2) as fpv,
                  tc.tile_pool(name="fwd2", bufs=4) as fp2,
                  tc.tile_pool(name="fwd_ps", bufs=4, space="PSUM") as fps):
                asp = fp_.tile([128, 2 * KC, DH], bf16)    # kernel spectrum
                # A: Ar/Ai m-tiles, contraction over L lags only
                for m in range(2 * KC - 1):
                    ps = fps.tile([128, DH], f32, name="aps", tag="aps")
                    for kc in range(LC):
                        nc.tensor.matmul(
                            ps[:], wd_sb[:, kc, m * 128:(m + 1) * 128],
                            acoef[:, kc, :], start=(kc == 0),
                            stop=(kc == LC - 1))
                    nc.scalar.activation(asp[:, m, :], ps[:], AFT.Copy)
                for b in range(B):
                    vb = fpv.tile([128, N // 128, DH], bf16, name="vb",
                                  tag="vb")
                    nc.sync.dma_start(
                        vb[:], v_d[:, b * (N // 128):(b + 1) * (N // 128), :])
                    xsp = fpx.tile([128, 2 * KC, DH], bf16, name="xsp",
                                   tag="xsp")
                    # D: X = DFT(v_b); interleave re/im pairs for E
                    for j in range(KC):
                        ms = (j,) if j == KC - 1 else (j, KC + j)
                        for m in ms:
                            ps = fps.tile([128, DH], f32, name="dps",
                                          tag="dps")
                            for kc in range(N // 128):
                                nc.tensor.matmul(
                                    ps[:],
                                    wd_sb[:, kc, m * 128:(m + 1) * 128],
                                    vb[:, kc, :], start=(kc == 0),
                                    stop=(kc == N // 128 - 1))
                            nc.scalar.activation(xsp[:, m, :], ps[:],
                                                 AFT.Copy)
                        # E: P = A * X (complex), in place over xsp
                        xr, xi = xsp[:, j, :], xsp[:, KC + j, :]
                        ar, ai = asp[:, j, :], asp[:, KC + j, :]
                        if j == KC - 1:
                            nc.vector.tensor_mul(xr, ar, xr)
                            nc.sync.dma_start(
                                psp_d[:, b * 2 * KC + j, :], xr)
                            continue
                        t1 = fp2.tile([128, DH], bf16, name="t1", tag="t1")
                        t2 = fp2.tile([128, DH], bf16, name="t2", tag="t2")
                        t3 = fp2.tile([128, DH], bf16, name="t3", tag="t3")
                        nc.vector.tensor_mul(t1[:], ar, xr)
                        nc.vector.tensor_mul(t2[:], ai, xi)
                        nc.vector.tensor_mul(t3[:], ar, xi)
                        nc.vector.tensor_mul(xi, ai, xr)
                        nc.vector.tensor_sub(xr, t1[:], t2[:])
                        nc.vector.tensor_add(xi, t3[:], xi)
                        nc.sync.dma_start(psp_d[:, b * 2 * KC + j, :], xr)
                        nc.sync.dma_start(
                            psp_d[:, b * 2 * KC + KC + j, :], xi)
            wd_pool.__exit__(None, None, None)

            # ---------------- inverse DFT + gate + o-projection ---------
            with (tc.tile_pool(name="wf", bufs=1) as wfp,
                  tc.tile_pool(name="inv", bufs=2) as ip_,
                  tc.tile_pool(name="invg", bufs=2) as gp_,
                  tc.tile_pool(name="invw", bufs=1) as owp,
                  tc.tile_pool(name="invs", bufs=4) as isp,
                  tc.tile_pool(name="inv_ps", bufs=4, space="PSUM") as ips):
                NJ = 2 * KC - 1        # Nyquist-sin chunk dropped
                wf_sb = wfp.tile([128, NJ, N], bf16)
                for j in range(NJ):    # per-chunk: F pipelines with the load
                    nc.scalar.dma_start(wf_sb[:, j, :], wf_d[:, j, :])
                ow_sb = owp.tile([128, DH // 128, E], bf16)
                nc.sync.dma_start(ow_sb[:], o_w3[:])
                for b in range(B):
                    pb = ip_.tile([128, NJ, DH], bf16, name="pb", tag="pb")
                    ub = ip_.tile([128, DH // 128, N], bf16, name="ub",
                                  tag="ub")
                    nc.sync.dma_start(
                        pb[:], psp_d[:, b * 2 * KC:b * 2 * KC + NJ, :])
                    nc.sync.dma_start(
                        ub[:], uT_d[:, :, b * N:(b + 1) * N])
                    gb = gp_.tile([128, DH // 128, N], bf16, name="gb",
                                  tag="gb")
                    # F: tv^T = sum_k P * WF ; gate with u in the evict
                    for m in range(DH // 128):
                        for tch in range(N // 512):
                            ps = ips.tile([128, 512], f32, name="fps",
                                          tag="fps")
                            for j in range(NJ):
                                nc.tensor.matmul(
                                    ps[:], pb[:, j, m * 128:(m + 1) * 128],
                                    wf_sb[:, j, tch * 512:(tch + 1) * 512],
                                    start=(j == 0), stop=(j == NJ - 1))
                            sl = slice(tch * 512, (tch + 1) * 512)
                            nc.vector.tensor_mul(
                                gb[:, m, sl], ps[:], ub[:, m, sl])
                    # H: partial o-projection out_b = g^T @ o_w
                    for mo in range(N // 128):
                        for ech in range(E // 512):
                            ps = ips.tile([128, 512], f32, name="hps",
                                          tag="hps")
                            for kc in range(DH // 128):
                                nc.tensor.matmul(
                                    ps[:], gb[:, kc, mo * 128:(mo + 1) * 128],
                                    ow_sb[:, kc, ech * 512:(ech + 1) * 512],
                                    start=(kc == 0), stop=(kc == DH // 128 - 1))
                            ot = isp.tile([128, 512], f32, name="ot",
                                          tag="ot")
                            nc.scalar.activation(ot[:], ps[:], AFT.Copy)
                            nc.sync.dma_start(
                                out[:, b * (N // 128) + mo,
                                    ech * 512:(ech + 1) * 512], ot[:])

    nc.compile()
    return nc


def _get_nc():
    if "nc" not in _CACHE:
        _CACHE["nc"] = _build()
    return _CACHE["nc"]


def kernel(x, u_w, u_b, v_w, v_b, o_w, o_b,
           pos_w, pos_b, lw0, lb0, lw1, lb1, lw2, lb2, out_w, out_b):
    from concourse.bass_utils import run_bass_kernel_spmd

    wd3, wf3, decay_t = _consts()
    x_flat = np.asarray(x, np.float32).reshape(ROWS, E)
    xTa = np.zeros((KA, ROWS), np.float32)
    xTa[:E] = x_flat.T
    xTa[E] = 1.0
    xTa3 = _t3(xTa, bfl)

    p_aug = np.stack([np.arange(L, dtype=np.float32),
                      np.ones(L, np.float32)])
    pw_aug = np.concatenate([pos_w, pos_b[None, :]], 0).astype(np.float32)
    lbs = np.concatenate(
        [bb.reshape(R // 128, 128).T for bb in (lb0, lb1, lb2)],
        axis=1).astype(np.float32)

    in_maps = []
    for h in range(H):
        sl = slice(h * DH, (h + 1) * DH)
        u_wa = np.zeros((KA, DH), np.float32)
        u_wa[:E] = u_w[:, sl]
        u_wa[E] = u_b[sl]
        v_wa = np.zeros((KA, DH), np.float32)
        v_wa[:E] = v_w[:, sl]
        v_wa[E] = v_b[sl]
        in_maps.append(dict(
            xTa=xTa3, u_wa=_t3(u_wa, bfl), v_wa=_t3(v_wa, bfl),
            o_w3=_t3(np.ascontiguousarray(o_w[sl, :]).astype(np.float32), bfl),
            wd=wd3, wf=wf3,
            p_aug=p_aug, pw_aug=pw_aug,
            lw0=_t3(lw0, bfl), lw1=_t3(lw1, bfl), lw2=_t3(lw2, bfl), lbs=lbs,
            out_w3=_t3(np.ascontiguousarray(out_w[:, sl]), bfl),
            outb=np.ascontiguousarray(out_b[None, sl]).astype(np.float32),
            decay=decay_t,
        ))

    nc = _get_nc()
    res = run_bass_kernel_spmd(nc, in_maps, core_ids=list(range(8)),
                               trace=bool(_CACHE.get("trace")))
    _CACHE["last_res"] = res
    acc = np.zeros((ROWS, E), np.float32)
    for i in range(H):
        acc += _from3(res.results[i]["out"])
    acc += o_b[None, :]
    return acc.reshape(B, N, E)


# revision 6
# speedup vs baseline: 23405.0177x; 1.0415x over previous
"""GTU (gated Toeplitz unit) Bass kernel for 8 TRN2 NeuronCores — v2.

Sharding: tensor-parallel over heads (H=8 -> 1 head/core); host sums the
8 partial o-projections.

vs the fp32 baseline (4.98 ms -> 0.94 ms simulated):
- All matmuls in bf16 (1 PE cycle/row vs 4 for fp32); fp32 PSUM
  accumulation; norm math in the RPE MLP stays fp32.
- Kernel lags truncated at L=640 (decay gamma^640 ~ 1.6e-3), shrinking
  the circular conv from 4096 to M2=2688 points and the RPE MLP to the
  640 positions that survive the decay.
- One SBUF-resident DFT matrix per phase, loaded once (not per batch)
  and prefetched on the second (Activation) DMA queue; forward spectra,
  complex multiply and gate all stay on-chip; only the P spectrum
  round-trips DRAM between the two DFT phases. With M2=2688 the
  Nyquist bin sits mid-chunk, so all 2*KC chunks are live.
- u/v projections fused into one pass streaming x^T tiles used as both
  moving (uT) and stationary (v) matmul operands, overlapped with the
  serial MLP chains to keep the PE fed.
"""

import numpy as np
import ml_dtypes

B, N, E = 4, 2048, 1024
H = 8
D1 = 3 * E
DH = D1 // H            # 384
R = 512
GAMMA = 0.99
EPS = 1e-8
L = 640                 # truncated kernel lags (5*128)
LC = L // 128           # 5
M2 = 2688               # circular conv length >= N + L - 1
KH = M2 // 2 + 1        # 1345 rfft bins
KC = 11                 # freq chunks of 128 (pad 1345 -> 1408)
KP = KC * 128           # 1408
ROWS = B * N            # 8192
KA = 1152               # augmented contraction for x (bias row), 9*128

_CACHE = {}

bfl = ml_dtypes.bfloat16


def _t3(a, dtype=np.float32):
    """(M, N) -> (128, M/128, N) partition-tiled layout."""
    m, n = a.shape
    assert m % 128 == 0
    return np.ascontiguousarray(
        a.reshape(m // 128, 128, n).transpose(1, 0, 2)).astype(dtype)


def _from3(a):
    p, m, n = a.shape
    return np.ascontiguousarray(
        np.asarray(a, np.float32).transpose(1, 0, 2)).reshape(m * 128, n)


def _consts():
    if "dft" in _CACHE:
        return _CACHE["dft"]
    t = np.arange(N, dtype=np.float64)[:, None]
    k = np.arange(KP, dtype=np.float64)[None, :]
    mask = (k <= (KH - 1)).astype(np.float64)
    ang = 2.0 * np.pi * t * k / M2
    cr = np.cos(ang) * mask
    ci = -np.sin(ang) * mask
    wd = np.concatenate([cr, ci], axis=1)                 # (2048, 3072)

    kk = np.arange(KP, dtype=np.float64)[:, None]
    tt = np.arange(N, dtype=np.float64)[None, :]
    w = np.where((kk == 0) | (kk == M2 // 2), 1.0, 2.0) * (kk <= (KH - 1)) / M2
    ang2 = 2.0 * np.pi * kk * tt / M2
    icos = w * np.cos(ang2)                               # (1536, 2048)
    isin = -w * np.sin(ang2)
    wf = np.concatenate([icos, isin], axis=0)             # (3072, 2048)

    decay = GAMMA ** np.arange(L, dtype=np.float64)       # lag 0 -> 1.0
    decay_t = decay.reshape(LC, 128).T                    # (128, 6)
    _CACHE["dft"] = (_t3(wd, bfl), _t3(wf, bfl), decay_t.astype(np.float32))
    return _CACHE["dft"]


def _build():
    import concourse.bass as bass
    import concourse.mybir as mybir
    import concourse.tile as tile
    from concourse import bacc

    AFT = mybir.ActivationFunctionType
    ALU = mybir.AluOpType
    f32 = mybir.dt.float32
    f32r = mybir.dt.float32r
    bf16 = mybir.dt.bfloat16

    nc = bacc.Bacc(None, target_bir_lowering=False, debug=False, num_devices=8)

    def din(name, shape, dt=f32):
        return nc.dram_tensor(name, list(shape), dt, kind="ExternalInput")

    def dint(name, shape, dt=bf16):
        return nc.dram_tensor(name, list(shape), dt)

    # inputs
    xTa = din("xTa", (128, KA // 128, ROWS), bf16)
    u_wa = din("u_wa", (128, KA // 128, DH), bf16)
    v_wa = din("v_wa", (128, KA // 128, DH), bf16)
    o_w3 = din("o_w3", (128, DH // 128, E), bf16)
    wd_d = din("wd", (128, N // 128, 2 * KP), bf16)
    wf_d = din("wf", (128, 2 * KC, N), bf16)
    p_aug = din("p_aug", (2, L))
    pw_aug = din("pw_aug", (2, R))
    lws = [din(f"lw{i}", (128, R // 128, R), bf16) for i in range(3)]
    lbs = din("lbs", (128, 3 * (R // 128)))
    out_w3 = din("out_w3", (128, R // 128, DH), bf16)
    outb = din("outb", (1, DH))
    decay = din("decay", (128, LC))
    out = nc.dram_tensor("out", [128, ROWS // 128, E], f32,
                         kind="ExternalOutput")

    # dram temps (bf16)
    uT_d = dint("uT_d", (128, DH // 128, ROWS))
    v_d = dint("v_d", (128, ROWS // 128, DH))
    psp_d = dint("psp_d", (128, B * 2 * KC, DH))

    FG = R // 128             # 4 feature groups
    PC = 320                  # MLP position-chunk width
    NCH = L // PC             # 2 position chunks in the (truncated) MLP

    with tile.TileContext(nc) as tc, nc.allow_low_precision(
            reason="bf16 pipeline validated against fp32 reference"):
        with tc.tile_pool(name="persist", bufs=1) as pp:
            acoef = pp.tile([128, LC, DH], bf16)   # truncated decayed coefs

            # wd is loaded up front: its pool sits above the phase-0/1
            # working set, so the 12.6MB DMA overlaps the MLP + u/v phase
            # instead of stalling the DFT phase behind it.
            wd_pool = tc.tile_pool(name="wd", bufs=1)
            wdp = wd_pool.__enter__()
            wd_sb = wdp.tile([128, N // 128, 2 * KP], bf16)
            for kc in range(N // 128):
                nc.scalar.dma_start(wd_sb[:, kc, :], wd_d[:, kc, :])

            # ------- RPE MLP + u/v projections (concurrent engines) -----
            # The MLP's serial norm->matmul chains leave the PE idle; the
            # u/v GEMMs stream through the same window and fill it.
            with (tc.tile_pool(name="mlp", bufs=1) as mp,
                  tc.tile_pool(name="mlp_ps", bufs=1, space="PSUM") as mps,
                  tc.tile_pool(name="uvw", bufs=1) as wp,
                  tc.tile_pool(name="uvx", bufs=3) as xp,
                  tc.tile_pool(name="uvs", bufs=4) as sp,
                  tc.tile_pool(name="uv_ps", bufs=2, space="PSUM") as ups):
                uw_sb = wp.tile([128, KA // 128, DH], bf16)
                vw_sb = wp.tile([128, KA // 128, DH], bf16)
                nc.sync.dma_start(uw_sb[:], u_wa[:])
                nc.sync.dma_start(vw_sb[:], v_wa[:])
                for grp in range(ROWS // 512):
                    xt = xp.tile([128, KA // 128, 512], bf16, name="xt",
                                 tag="xt")
                    nc.sync.dma_start(
                        xt[:], xTa[:, :, grp * 512:(grp + 1) * 512])
                    # uT tile: out[M=DH, N=512 rows]
                    for m in range(DH // 128):
                        ps = ups.tile([128, 512], f32, name="bps", tag="bps")
                        for kc in range(KA // 128):
                            nc.tensor.matmul(
                                ps[:], uw_sb[:, kc, m * 128:(m + 1) * 128],
                                xt[:, kc, :], start=(kc == 0),
                                stop=(kc == KA // 128 - 1))
                        ut = sp.tile([128, 512], bf16, name="ut", tag="ut")
                        nc.scalar.activation(ut[:], ps[:], AFT.Silu)
                        nc.sync.dma_start(
                            uT_d[:, m, grp * 512:(grp + 1) * 512], ut[:])
                    # v tiles: out[M=128 rows, N=DH]
                    for rs in range(4):
                        ps = ups.tile([128, DH], f32, name="cps", tag="cps")
                        for kc in range(KA // 128):
                            nc.tensor.matmul(
                                ps[:], xt[:, kc, rs * 128:(rs + 1) * 128],
                                vw_sb[:, kc, :], start=(kc == 0),
                                stop=(kc == KA // 128 - 1))
                        vt = sp.tile([128, DH], bf16, name="vt", tag="vt")
                        nc.scalar.activation(vt[:], ps[:], AFT.Silu)
                        nc.sync.dma_start(v_d[:, grp * 4 + rs, :], vt[:])
                ones_col = mp.tile([128, 1], bf16)     # K=128 -> M=1 reducer
                nc.vector.memset(ones_col[:], 1.0)
                one_row = mp.tile([1, 128], bf16)      # K=1 -> 128-part bcast
                nc.vector.memset(one_row[:], 1.0)
                one_rowf = mp.tile([1, 128], f32)
                nc.vector.memset(one_rowf[:], 1.0)
                c_sc = mp.tile([1, 1], f32)
                nc.vector.memset(c_sc[:], float(R ** -0.5))
                eps_sc = mp.tile([1, 1], f32)
                nc.vector.memset(eps_sc[:], EPS)

                pa_sb = mp.tile([2, L], f32)
                pw_sb = mp.tile([2, R], f32)
                lb_sb = mp.tile([128, 3 * FG], f32)
                nc.sync.dma_start(pa_sb[:], p_aug[:])
                nc.sync.dma_start(pw_sb[:], pw_aug[:])
                nc.sync.dma_start(lb_sb[:], lbs[:])

                # MLP runs only on the L kept lags; fp32 h, bf16 matmuls
                h = [mp.tile([128, L], f32, name=f"h{g}", tag=f"h{g}")
                     for g in range(FG)]
                # h0 = pos_idx @ pos_w + pos_b (K=2, fp32), feature-major
                for g in range(FG):
                    for nch in range(NCH):
                        ps = mps.tile([128, PC], f32, name="mmps", tag="mm")
                        nc.tensor.matmul(
                            ps[:], pw_sb[:, g * 128:(g + 1) * 128],
                            pa_sb[:, nch * PC:(nch + 1) * PC],
                            start=True, stop=True)
                        nc.vector.tensor_copy(
                            h[g][:, nch * PC:(nch + 1) * PC], ps[:])

                def srms_relu(h_in, phi_out):
                    # s[t] = sum_f h^2 ; factor = 1/(sqrt(s)/sqrt(R) + eps)
                    sq = [mp.tile([128, L], bf16, name=f"sq{g}", tag=f"sq{g}")
                          for g in range(FG)]
                    for g in range(FG):
                        nc.vector.tensor_mul(sq[g][:], h_in[g][:], h_in[g][:])
                    facb = mp.tile([1, L], bf16, name="facb", tag="facb")
                    fac = mp.tile([1, L], f32, name="fac", tag="fac")
                    for nch in range(NCH):
                        ps1 = mps.tile([1, PC], f32, name="redps", tag="red")
                        for g in range(FG):
                            nc.tensor.matmul(
                                ps1[:], ones_col[:],
                                sq[g][:, nch * PC:(nch + 1) * PC],
                                start=(g == 0), stop=(g == FG - 1))
                        sl = fac[:, nch * PC:(nch + 1) * PC]
                        nc.scalar.activation(sl, ps1[:], AFT.Sqrt)
                        nc.vector.tensor_scalar(
                            sl, sl, c_sc[:], eps_sc[:], ALU.mult, ALU.add)
                        nc.vector.reciprocal(
                            facb[:, nch * PC:(nch + 1) * PC], sl)
                    fb = mp.tile([128, L], f32, name="fb", tag="fb")
                    for nch in range(NCH):
                        psb = mps.tile([128, PC], f32, name="bcps", tag="bc")
                        nc.tensor.matmul(
                            psb[:], one_row[:],
                            facb[:, nch * PC:(nch + 1) * PC],
                            start=True, stop=True)
                        nc.vector.tensor_copy(
                            fb[:, nch * PC:(nch + 1) * PC], psb[:])
                    for g in range(FG):
                        nc.vector.tensor_mul(phi_out[g][:], h_in[g][:], fb[:])
                        nc.scalar.activation(
                            phi_out[g][:], phi_out[g][:], AFT.Relu)

                phi = [mp.tile([128, L], bf16, name=f"phi{g}", tag=f"phi{g}")
                       for g in range(FG)]
                srms_relu(h, phi)

                lw_sb = mp.tile([128, FG, R], bf16)
                for li in range(3):
                    nc.sync.dma_start(lw_sb[:], lws[li][:])
                    for g in range(FG):
                        for nch in range(NCH):
                            ps = mps.tile([128, PC], f32, name="mmps",
                                          tag="mm")
                            for kk in range(FG):
                                nc.tensor.matmul(
                                    ps[:],
                                    lw_sb[:, kk, g * 128:(g + 1) * 128],
                                    phi[kk][:, nch * PC:(nch + 1) * PC],
                                    start=(kk == 0), stop=(kk == FG - 1))
                            sl = h[g][:, nch * PC:(nch + 1) * PC]
                            nc.vector.tensor_scalar(
                                sl, ps[:],
                                lb_sb[:, li * FG + g:li * FG + g + 1],
                                None, ALU.add)
                    srms_relu(h, phi)

                # coefs (t-major, lags < L only) = phi.T @ out_w
                ow_sb = mp.tile([128, FG, DH], bf16)
                ob_sb = mp.tile([1, DH], f32)
                dec_sb = mp.tile([128, LC], f32)
                nc.sync.dma_start(ow_sb[:], out_w3[:])
                nc.sync.dma_start(ob_sb[:], outb[:])
                nc.sync.dma_start(dec_sb[:], decay[:])
                obb = mp.tile([128, DH], f32)
                psb = mps.tile([128, DH], f32, name="bc2ps", tag="bc")
                nc.tensor.matmul(psb[:], one_rowf[:], ob_sb[:],
                                 start=True, stop=True)
                nc.vector.tensor_copy(obb[:], psb[:])
                for m in range(LC):
                    ps = mps.tile([128, DH], f32, name="mm2ps", tag="mm")
                    for kk in range(FG):
                        nc.tensor.matmul(
                            ps[:], phi[kk][:, m * 128:(m + 1) * 128],
                            ow_sb[:, kk, :], start=(kk == 0),
                            stop=(kk == FG - 1))
                    ac = mp.tile([128, DH], f32, name="ac", tag="ac")
                    nc.vector.tensor_add(ac[:], ps[:], obb[:])
                    nc.vector.tensor_scalar(
                        acoef[:, m, :], ac[:], dec_sb[:, m:m + 1],
                        None, ALU.mult)

            # ---------------- forward DFTs + complex multiply -----------
            with (tc.tile_pool(name="fwd", bufs=1) as fp_,
                  tc.tile_pool(name="fwdx", bufs=2) as fpx,
                  tc.tile_pool(name="fwdv", bufs=2) as fpv,
                  tc.tile_pool(name="fwd2", bufs=4) as fp2,
                  tc.tile_pool(name="fwd_ps", bufs=4, space="PSUM") as fps):
                asp = fp_.tile([128, 2 * KC, DH], bf16)    # kernel spectrum
                # A: Ar/Ai m-tiles, contraction over L lags only
                for m in range(2 * KC):
                    ps = fps.tile([128, DH], f32, name="aps", tag="aps")
                    for kc in range(LC):
                        nc.tensor.matmul(
                            ps[:], wd_sb[:, kc, m * 128:(m + 1) * 128],
                            acoef[:, kc, :], start=(kc == 0),
                            stop=(kc == LC - 1))
                    nc.scalar.activation(asp[:, m, :], ps[:], AFT.Copy)
                for b in range(B):
                    vb = fpv.tile([128, N // 128, DH], bf16, name="vb",
                                  tag="vb")
                    nc.sync.dma_start(
                        vb[:], v_d[:, b * (N // 128):(b + 1) * (N // 128), :])
                    xsp = fpx.tile([128, 2 * KC, DH], bf16, name="xsp",
                                   tag="xsp")
                    # D: X = DFT(v_b); interleave re/im pairs for E
                    for j in range(KC):
                        for m in (j, KC + j):
                            ps = fps.tile([128, DH], f32, name="dps",
                                          tag="dps")
                            for kc in range(N // 128):
                                nc.tensor.matmul(
                                    ps[:],
                                    wd_sb[:, kc, m * 128:(m + 1) * 128],
                                    vb[:, kc, :], start=(kc == 0),
                                    stop=(kc == N // 128 - 1))
                            nc.scalar.activation(xsp[:, m, :], ps[:],
                                                 AFT.Copy)
                        # E: P = A * X (complex), in place over xsp
                        xr, xi = xsp[:, j, :], xsp[:, KC + j, :]
                        ar, ai = asp[:, j, :], asp[:, KC + j, :]
                        t1 = fp2.tile([128, DH], bf16, name="t1", tag="t1")
                        t2 = fp2.tile([128, DH], bf16, name="t2", tag="t2")
                        t3 = fp2.tile([128, DH], bf16, name="t3", tag="t3")
                        nc.vector.tensor_mul(t1[:], ar, xr)
                        nc.vector.tensor_mul(t2[:], ai, xi)
                        nc.vector.tensor_mul(t3[:], ar, xi)
                        nc.vector.tensor_mul(xi, ai, xr)
                        nc.vector.tensor_sub(xr, t1[:], t2[:])
                        nc.vector.tensor_add(xi, t3[:], xi)
                        nc.sync.dma_start(psp_d[:, b * 2 * KC + j, :], xr)
                        nc.sync.dma_start(
                            psp_d[:, b * 2 * KC + KC + j, :], xi)
            wd_pool.__exit__(None, None, None)

            # ---------------- inverse DFT + gate + o-projection ---------
            with (tc.tile_pool(name="wf", bufs=1) as wfp,
                  tc.tile_pool(name="inv", bufs=2) as ip_,
                  tc.tile_pool(name="invg", bufs=2) as gp_,
                  tc.tile_pool(name="invw", bufs=1) as owp,
                  tc.tile_pool(name="invs", bufs=4) as isp,
                  tc.tile_pool(name="inv_ps", bufs=4, space="PSUM") as ips):
                NJ = 2 * KC
                wf_sb = wfp.tile([128, NJ, N], bf16)
                for j in range(NJ):    # per-chunk: F pipelines with the load
                    nc.scalar.dma_start(wf_sb[:, j, :], wf_d[:, j, :])
                ow_sb = owp.tile([128, DH // 128, E], bf16)
                nc.sync.dma_start(ow_sb[:], o_w3[:])
                for b in range(B):
                    pb = ip_.tile([128, NJ, DH], bf16, name="pb", tag="pb")
                    ub = ip_.tile([128, DH // 128, N], bf16, name="ub",
                                  tag="ub")
                    nc.sync.dma_start(
                        pb[:], psp_d[:, b * 2 * KC:b * 2 * KC + NJ, :])
                    nc.sync.dma_start(
                        ub[:], uT_d[:, :, b * N:(b + 1) * N])
                    gb = gp_.tile([128, DH // 128, N], bf16, name="gb",
                                  tag="gb")
                    # F: tv^T = sum_k P * WF ; gate with u in the evict
                    for m in range(DH // 128):
                        for tch in range(N // 512):
                            ps = ips.tile([128, 512], f32, name="fps",
                                          tag="fps")
                            for j in range(NJ):
                                nc.tensor.matmul(
                                    ps[:], pb[:, j, m * 128:(m + 1) * 128],
                                    wf_sb[:, j, tch * 512:(tch + 1) * 512],
                                    start=(j == 0), stop=(j == NJ - 1))
                            sl = slice(tch * 512, (tch + 1) * 512)
                            nc.vector.tensor_mul(
                                gb[:, m, sl], ps[:], ub[:, m, sl])
                    # H: partial o-projection out_b = g^T @ o_w
                    for mo in range(N // 128):
                        for ech in range(E // 512):
                            ps = ips.tile([128, 512], f32, name="hps",
                                          tag="hps")
                            for kc in range(DH // 128):
                                nc.tensor.matmul(
                                    ps[:], gb[:, kc, mo * 128:(mo + 1) * 128],
                                    ow_sb[:, kc, ech * 512:(ech + 1) * 512],
                                    start=(kc == 0), stop=(kc == DH // 128 - 1))
                            ot = isp.tile([128, 512], f32, name="ot",
                                          tag="ot")
                            nc.scalar.activation(ot[:], ps[:], AFT.Copy)
                            nc.sync.dma_start(
                                out[:, b * (N // 128) + mo,
                                    ech * 512:(ech + 1) * 512], ot[:])

    nc.compile()
    return nc


def _get_nc():
    if "nc" not in _CACHE:
        _CACHE["nc"] = _build()
    return _CACHE["nc"]


def kernel(x, u_w, u_b, v_w, v_b, o_w, o_b,
           pos_w, pos_b, lw0, lb0, lw1, lb1, lw2, lb2, out_w, out_b):
    from concourse.bass_utils import run_bass_kernel_spmd

    wd3, wf3, decay_t = _consts()
    x_flat = np.asarray(x, np.float32).reshape(ROWS, E)
    xTa = np.zeros((KA, ROWS), np.float32)
    xTa[:E] = x_flat.T
    xTa[E] = 1.0
    xTa3 = _t3(xTa, bfl)

    p_aug = np.stack([np.arange(L, dtype=np.float32),
                      np.ones(L, np.float32)])
    pw_aug = np.concatenate([pos_w, pos_b[None, :]], 0).astype(np.float32)
    lbs = np.concatenate(
        [bb.reshape(R // 128, 128).T for bb in (lb0, lb1, lb2)],
        axis=1).astype(np.float32)

    in_maps = []
    for h in range(H):
        sl = slice(h * DH, (h + 1) * DH)
        u_wa = np.zeros((KA, DH), np.float32)
        u_wa[:E] = u_w[:, sl]
        u_wa[E] = u_b[sl]
        v_wa = np.zeros((KA, DH), np.float32)
        v_wa[:E] = v_w[:, sl]
        v_wa[E] = v_b[sl]
        in_maps.append(dict(
            xTa=xTa3, u_wa=_t3(u_wa, bfl), v_wa=_t3(v_wa, bfl),
            o_w3=_t3(np.ascontiguousarray(o_w[sl, :]).astype(np.float32), bfl),
            wd=wd3, wf=wf3,
            p_aug=p_aug, pw_aug=pw_aug,
            lw0=_t3(lw0, bfl), lw1=_t3(lw1, bfl), lw2=_t3(lw2, bfl), lbs=lbs,
            out_w3=_t3(np.ascontiguousarray(out_w[:, sl]), bfl),
            outb=np.ascontiguousarray(out_b[None, sl]).astype(np.float32),
            decay=decay_t,
        ))

    nc = _get_nc()
    res = run_bass_kernel_spmd(nc, in_maps, core_ids=list(range(8)),
                               trace=bool(_CACHE.get("trace")))
    _CACHE["last_res"] = res
    acc = np.zeros((ROWS, E), np.float32)
    for i in range(H):
        acc += _from3(res.results[i]["out"])
    acc += o_b[None, :]
    return acc.reshape(B, N, E)


# revision 7
# speedup vs baseline: 24747.4578x; 1.0574x over previous
"""GTU (gated Toeplitz unit) Bass kernel for 8 TRN2 NeuronCores — v2.

Sharding: tensor-parallel over heads (H=8 -> 1 head/core); host sums the
8 partial o-projections.

vs the fp32 baseline (4.98 ms -> 0.94 ms simulated):
- All matmuls in bf16 (1 PE cycle/row vs 4 for fp32); fp32 PSUM
  accumulation; norm math in the RPE MLP stays fp32.
- Kernel lags truncated at L=640 (decay gamma^640 ~ 1.6e-3), shrinking
  the circular conv from 4096 to M2=2688 points and the RPE MLP to the
  640 positions that survive the decay.
- One SBUF-resident DFT matrix per phase, loaded once (not per batch)
  and prefetched on the second (Activation) DMA queue; forward spectra,
  complex multiply and gate all stay on-chip; only the P spectrum
  round-trips DRAM between the two DFT phases. With M2=2688 the
  Nyquist bin sits mid-chunk, so all 2*KC chunks are live.
- u/v projections fused into one pass streaming x^T tiles used as both
  moving (uT) and stationary (v) matmul operands, overlapped with the
  serial MLP chains to keep the PE fed.
"""

import numpy as np
import ml_dtypes

B, N, E = 4, 2048, 1024
H = 8
D1 = 3 * E
DH = D1 // H            # 384
R = 512
GAMMA = 0.99
EPS = 1e-8
L = 640                 # truncated kernel lags (5*128)
LC = L // 128           # 5
M2 = 2688               # circular conv length >= N + L - 1
KH = M2 // 2 + 1        # 1345 rfft bins
KC = 11                 # freq chunks of 128 (pad 1345 -> 1408)
KP = KC * 128           # 1408
ROWS = B * N            # 8192
KA = 1152               # augmented contraction for x (bias row), 9*128

_CACHE = {}

bfl = ml_dtypes.bfloat16


def _t3(a, dtype=np.float32):
    """(M, N) -> (128, M/128, N) partition-tiled layout."""
    m, n = a.shape
    assert m % 128 == 0
    return np.ascontiguousarray(
        a.reshape(m // 128, 128, n).transpose(1, 0, 2)).astype(dtype)


def _from3(a):
    p, m, n = a.shape
    return np.ascontiguousarray(
        np.asarray(a, np.float32).transpose(1, 0, 2)).reshape(m * 128, n)


def _consts():
    if "dft" in _CACHE:
        return _CACHE["dft"]
    t = np.arange(N, dtype=np.float64)[:, None]
    k = np.arange(KP, dtype=np.float64)[None, :]
    mask = (k <= (KH - 1)).astype(np.float64)
    ang = 2.0 * np.pi * t * k / M2
    cr = np.cos(ang) * mask
    ci = -np.sin(ang) * mask
    wd = np.concatenate([cr, ci], axis=1)                 # (2048, 3072)

    kk = np.arange(KP, dtype=np.float64)[:, None]
    tt = np.arange(N, dtype=np.float64)[None, :]
    w = np.where((kk == 0) | (kk == M2 // 2), 1.0, 2.0) * (kk <= (KH - 1)) / M2
    ang2 = 2.0 * np.pi * kk * tt / M2
    icos = w * np.cos(ang2)                               # (1536, 2048)
    isin = -w * np.sin(ang2)
    wf = np.concatenate([icos, isin], axis=0)             # (3072, 2048)

    decay = GAMMA ** np.arange(L, dtype=np.float64)       # lag 0 -> 1.0
    decay_t = decay.reshape(LC, 128).T                    # (128, 6)
    _CACHE["dft"] = (_t3(wd, bfl), _t3(wf, bfl), decay_t.astype(np.float32))
    return _CACHE["dft"]


def _build():
    import concourse.bass as bass
    import concourse.mybir as mybir
    import concourse.tile as tile
    from concourse import bacc

    AFT = mybir.ActivationFunctionType
    ALU = mybir.AluOpType
    f32 = mybir.dt.float32
    f32r = mybir.dt.float32r
    bf16 = mybir.dt.bfloat16

    nc = bacc.Bacc(None, target_bir_lowering=False, debug=False, num_devices=8)

    def din(name, shape, dt=f32):
        return nc.dram_tensor(name, list(shape), dt, kind="ExternalInput")

    def dint(name, shape, dt=bf16):
        return nc.dram_tensor(name, list(shape), dt)

    # inputs
    xTa = din("xTa", (128, KA // 128, ROWS), bf16)
    u_wa = din("u_wa", (128, KA // 128, DH), bf16)
    v_wa = din("v_wa", (128, KA // 128, DH), bf16)
    o_w3 = din("o_w3", (128, DH // 128, E), bf16)
    wd_d = din("wd", (128, N // 128, 2 * KP), bf16)
    wf_d = din("wf", (128, 2 * KC, N), bf16)
    p_aug = din("p_aug", (2, L))
    pw_aug = din("pw_aug", (2, R))
    lws = [din(f"lw{i}", (128, R // 128, R), bf16) for i in range(3)]
    lbs = din("lbs", (128, 3 * (R // 128)))
    out_w3 = din("out_w3", (128, R // 128, DH), bf16)
    outb = din("outb", (1, DH))
    decay = din("decay", (128, LC))
    out = nc.dram_tensor("out", [128, ROWS // 128, E], f32,
                         kind="ExternalOutput")

    # dram temps (bf16)
    uT_d = dint("uT_d", (128, DH // 128, ROWS))
    v_d = dint("v_d", (128, ROWS // 128, DH))
    psp_d = dint("psp_d", (128, B * 2 * KC, DH))

    FG = R // 128             # 4 feature groups
    PC = 320                  # MLP position-chunk width
    NCH = L // PC             # 2 position chunks in the (truncated) MLP

    with tile.TileContext(nc) as tc, nc.allow_low_precision(
            reason="bf16 pipeline validated against fp32 reference"):
        with tc.tile_pool(name="persist", bufs=1) as pp:
            acoef = pp.tile([128, LC, DH], bf16)   # truncated decayed coefs

            # wd is loaded up front: its pool sits above the phase-0/1
            # working set, so the 12.6MB DMA overlaps the MLP + u/v phase
            # instead of stalling the DFT phase behind it.
            uw_sb = pp.tile([128, KA // 128, DH], bf16)
            vw_sb = pp.tile([128, KA // 128, DH], bf16)
            WFA = 4
            wfa_sb = pp.tile([128, WFA, N], bf16)
            nc.scalar.dma_start(uw_sb[:], u_wa[:])
            nc.scalar.dma_start(vw_sb[:], v_wa[:])
            wd_pool = tc.tile_pool(name="wd", bufs=1)
            wdp = wd_pool.__enter__()
            wd_sb = wdp.tile([128, N // 128, 2 * KP], bf16)
            for kc in range(N // 128):
                nc.scalar.dma_start(wd_sb[:, kc, :], wd_d[:, kc, :])

            # ------- RPE MLP + u/v projections (concurrent engines) -----
            # The MLP's serial norm->matmul chains leave the PE idle; the
            # u/v GEMMs stream through the same window and fill it.
            with (tc.tile_pool(name="mlp", bufs=1) as mp,
                  tc.tile_pool(name="mlp_ps", bufs=1, space="PSUM") as mps,
                  tc.tile_pool(name="uvx", bufs=3) as xp,
                  tc.tile_pool(name="uvs", bufs=4) as sp,
                  tc.tile_pool(name="uv_ps", bufs=2, space="PSUM") as ups):
                # Engines run their streams in order, so the serial MLP
                # chain must be INTERLEAVED with the u/v GEMM groups at
                # emission time or it just runs after them. uv_feed(k)
                # emits the next k groups; it is called between MLP
                # pipeline stages.
                uv_pending = list(range(ROWS // 512))

                def uv_group(grp):
                    xt = xp.tile([128, KA // 128, 512], bf16, name="xt",
                                 tag="xt")
                    nc.sync.dma_start(
                        xt[:], xTa[:, :, grp * 512:(grp + 1) * 512])
                    # uT tile: out[M=DH, N=512 rows]
                    for m in range(DH // 128):
                        ps = ups.tile([128, 512], f32, name="bps", tag="bps")
                        for kc in range(KA // 128):
                            nc.tensor.matmul(
                                ps[:], uw_sb[:, kc, m * 128:(m + 1) * 128],
                                xt[:, kc, :], start=(kc == 0),
                                stop=(kc == KA // 128 - 1))
                        ut = sp.tile([128, 512], bf16, name="ut", tag="ut")
                        nc.scalar.activation(ut[:], ps[:], AFT.Silu)
                        nc.sync.dma_start(
                            uT_d[:, m, grp * 512:(grp + 1) * 512], ut[:])
                    # v tiles: out[M=128 rows, N=DH]
                    for rs in range(4):
                        ps = ups.tile([128, DH], f32, name="cps", tag="cps")
                        for kc in range(KA // 128):
                            nc.tensor.matmul(
                                ps[:], xt[:, kc, rs * 128:(rs + 1) * 128],
                                vw_sb[:, kc, :], start=(kc == 0),
                                stop=(kc == KA // 128 - 1))
                        vt = sp.tile([128, DH], bf16, name="vt", tag="vt")
                        nc.scalar.activation(vt[:], ps[:], AFT.Silu)
                        nc.sync.dma_start(v_d[:, grp * 4 + rs, :], vt[:])

                def uv_feed(k):
                    for _ in range(min(k, len(uv_pending))):
                        uv_group(uv_pending.pop(0))

                uv_feed(1)
                ones_col = mp.tile([128, 1], bf16)     # K=128 -> M=1 reducer
                nc.vector.memset(ones_col[:], 1.0)
                one_row = mp.tile([1, 128], bf16)      # K=1 -> 128-part bcast
                nc.vector.memset(one_row[:], 1.0)
                one_rowf = mp.tile([1, 128], f32)
                nc.vector.memset(one_rowf[:], 1.0)
                c_sc = mp.tile([1, 1], f32)
                nc.vector.memset(c_sc[:], float(R ** -0.5))
                eps_sc = mp.tile([1, 1], f32)
                nc.vector.memset(eps_sc[:], EPS)

                pa_sb = mp.tile([2, L], f32)
                pw_sb = mp.tile([2, R], f32)
                lb_sb = mp.tile([128, 3 * FG], f32)
                nc.sync.dma_start(pa_sb[:], p_aug[:])
                nc.sync.dma_start(pw_sb[:], pw_aug[:])
                nc.sync.dma_start(lb_sb[:], lbs[:])

                # MLP runs only on the L kept lags; fp32 h, bf16 matmuls
                h = [mp.tile([128, L], f32, name=f"h{g}", tag=f"h{g}")
                     for g in range(FG)]
                # h0 = pos_idx @ pos_w + pos_b (K=2, fp32), feature-major
                for g in range(FG):
                    for nch in range(NCH):
                        ps = mps.tile([128, PC], f32, name="mmps", tag="mm")
                        nc.tensor.matmul(
                            ps[:], pw_sb[:, g * 128:(g + 1) * 128],
                            pa_sb[:, nch * PC:(nch + 1) * PC],
                            start=True, stop=True)
                        nc.vector.tensor_copy(
                            h[g][:, nch * PC:(nch + 1) * PC], ps[:])

                def srms_relu(h_in, phi_out):
                    # s[t] = sum_f h^2 ; factor = 1/(sqrt(s)/sqrt(R) + eps)
                    sq = [mp.tile([128, L], bf16, name=f"sq{g}", tag=f"sq{g}")
                          for g in range(FG)]
                    for g in range(FG):
                        nc.vector.tensor_mul(sq[g][:], h_in[g][:], h_in[g][:])
                    facb = mp.tile([1, L], bf16, name="facb", tag="facb")
                    fac = mp.tile([1, L], f32, name="fac", tag="fac")
                    for nch in range(NCH):
                        ps1 = mps.tile([1, PC], f32, name="redps", tag="red")
                        for g in range(FG):
                            nc.tensor.matmul(
                                ps1[:], ones_col[:],
                                sq[g][:, nch * PC:(nch + 1) * PC],
                                start=(g == 0), stop=(g == FG - 1))
                        sl = fac[:, nch * PC:(nch + 1) * PC]
                        nc.scalar.activation(sl, ps1[:], AFT.Sqrt)
                        nc.vector.tensor_scalar(
                            sl, sl, c_sc[:], eps_sc[:], ALU.mult, ALU.add)
                        nc.vector.reciprocal(
                            facb[:, nch * PC:(nch + 1) * PC], sl)
                    fb = mp.tile([128, L], f32, name="fb", tag="fb")
                    for nch in range(NCH):
                        psb = mps.tile([128, PC], f32, name="bcps", tag="bc")
                        nc.tensor.matmul(
                            psb[:], one_row[:],
                            facb[:, nch * PC:(nch + 1) * PC],
                            start=True, stop=True)
                        nc.vector.tensor_copy(
                            fb[:, nch * PC:(nch + 1) * PC], psb[:])
                    for g in range(FG):
                        nc.vector.tensor_mul(phi_out[g][:], h_in[g][:], fb[:])
                        nc.scalar.activation(
                            phi_out[g][:], phi_out[g][:], AFT.Relu)

                phi = [mp.tile([128, L], bf16, name=f"phi{g}", tag=f"phi{g}")
                       for g in range(FG)]
                uv_feed(1)
                srms_relu(h, phi)
                uv_feed(2)

                lw_sb = mp.tile([128, FG, R], bf16)
                for li in range(3):
                    nc.sync.dma_start(lw_sb[:], lws[li][:])
                    for g in range(FG):
                        for nch in range(NCH):
                            ps = mps.tile([128, PC], f32, name="mmps",
                                          tag="mm")
                            for kk in range(FG):
                                nc.tensor.matmul(
                                    ps[:],
                                    lw_sb[:, kk, g * 128:(g + 1) * 128],
                                    phi[kk][:, nch * PC:(nch + 1) * PC],
                                    start=(kk == 0), stop=(kk == FG - 1))
                            sl = h[g][:, nch * PC:(nch + 1) * PC]
                            nc.vector.tensor_scalar(
                                sl, ps[:],
                                lb_sb[:, li * FG + g:li * FG + g + 1],
                                None, ALU.add)
                    uv_feed(2)
                    srms_relu(h, phi)
                    uv_feed(2)

                # coefs (t-major, lags < L only) = phi.T @ out_w
                ow_sb = mp.tile([128, FG, DH], bf16)
                ob_sb = mp.tile([1, DH], f32)
                dec_sb = mp.tile([128, LC], f32)
                nc.sync.dma_start(ow_sb[:], out_w3[:])
                nc.sync.dma_start(ob_sb[:], outb[:])
                nc.sync.dma_start(dec_sb[:], decay[:])
                obb = mp.tile([128, DH], f32)
                psb = mps.tile([128, DH], f32, name="bc2ps", tag="bc")
                nc.tensor.matmul(psb[:], one_rowf[:], ob_sb[:],
                                 start=True, stop=True)
                nc.vector.tensor_copy(obb[:], psb[:])
                uv_feed(1)
                for m in range(LC):
                    uv_feed(1)
                    ps = mps.tile([128, DH], f32, name="mm2ps", tag="mm")
                    for kk in range(FG):
                        nc.tensor.matmul(
                            ps[:], phi[kk][:, m * 128:(m + 1) * 128],
                            ow_sb[:, kk, :], start=(kk == 0),
                            stop=(kk == FG - 1))
                    ac = mp.tile([128, DH], f32, name="ac", tag="ac")
                    nc.vector.tensor_add(ac[:], ps[:], obb[:])
                    nc.vector.tensor_scalar(
                        acoef[:, m, :], ac[:], dec_sb[:, m:m + 1],
                        None, ALU.mult)
                uv_feed(ROWS // 512)

            # wf head-piece: loads during phase 2 (persist pool, so no
            # dependence on wd's release), letting the inverse phase
            # start before the full wf load completes.
            for j in range(WFA):
                nc.scalar.dma_start(wfa_sb[:, j, :], wf_d[:, j, :])

            # ---------------- forward DFTs + complex multiply -----------
            with (tc.tile_pool(name="fwd", bufs=1) as fp_,
                  tc.tile_pool(name="fwdx", bufs=2) as fpx,
                  tc.tile_pool(name="fwdv", bufs=2) as fpv,
                  tc.tile_pool(name="fwd2", bufs=4) as fp2,
                  tc.tile_pool(name="fwd_ps", bufs=4, space="PSUM") as fps):
                asp = fp_.tile([128, 2 * KC, DH], bf16)    # kernel spectrum
                # A: Ar/Ai m-tiles, contraction over L lags only
                for m in range(2 * KC):
                    ps = fps.tile([128, DH], f32, name="aps", tag="aps")
                    for kc in range(LC):
                        nc.tensor.matmul(
                            ps[:], wd_sb[:, kc, m * 128:(m + 1) * 128],
                            acoef[:, kc, :], start=(kc == 0),
                            stop=(kc == LC - 1))
                    nc.scalar.activation(asp[:, m, :], ps[:], AFT.Copy)
                for b in range(B):
                    vb = fpv.tile([128, N // 128, DH], bf16, name="vb",
                                  tag="vb")
                    nc.sync.dma_start(
                        vb[:], v_d[:, b * (N // 128):(b + 1) * (N // 128), :])
                    xsp = fpx.tile([128, 2 * KC, DH], bf16, name="xsp",
                                   tag="xsp")
                    # D: X = DFT(v_b); interleave re/im pairs for E
                    for j in range(KC):
                        for m in (j, KC + j):
                            ps = fps.tile([128, DH], f32, name="dps",
                                          tag="dps")
                            for kc in range(N // 128):
                                nc.tensor.matmul(
                                    ps[:],
                                    wd_sb[:, kc, m * 128:(m + 1) * 128],
                                    vb[:, kc, :], start=(kc == 0),
                                    stop=(kc == N // 128 - 1))
                            nc.scalar.activation(xsp[:, m, :], ps[:],
                                                 AFT.Copy)
                        # E: P = A * X (complex), in place over xsp
                        xr, xi = xsp[:, j, :], xsp[:, KC + j, :]
                        ar, ai = asp[:, j, :], asp[:, KC + j, :]
                        t1 = fp2.tile([128, DH], bf16, name="t1", tag="t1")
                        t2 = fp2.tile([128, DH], bf16, name="t2", tag="t2")
                        t3 = fp2.tile([128, DH], bf16, name="t3", tag="t3")
                        nc.vector.tensor_mul(t1[:], ar, xr)
                        nc.vector.tensor_mul(t2[:], ai, xi)
                        nc.vector.tensor_mul(t3[:], ar, xi)
                        nc.vector.tensor_mul(xi, ai, xr)
                        nc.vector.tensor_sub(xr, t1[:], t2[:])
                        nc.vector.tensor_add(xi, t3[:], xi)
                        nc.sync.dma_start(psp_d[:, b * 2 * KC + j, :], xr)
                        nc.sync.dma_start(
                            psp_d[:, b * 2 * KC + KC + j, :], xi)
            wd_pool.__exit__(None, None, None)


            # ---------------- inverse DFT + gate + o-projection ---------
            with (tc.tile_pool(name="wf", bufs=1) as wfp,
                  tc.tile_pool(name="inv", bufs=2) as ip_,
                  tc.tile_pool(name="invg", bufs=2) as gp_,
                  tc.tile_pool(name="invw", bufs=1) as owp,
                  tc.tile_pool(name="invs", bufs=4) as isp,
                  tc.tile_pool(name="inv_ps", bufs=4, space="PSUM") as ips):
                NJ = 2 * KC
                wf_sb = wfp.tile([128, NJ - WFA, N], bf16)
                for j in range(NJ - WFA):  # tail: F pipelines with the load
                    nc.scalar.dma_start(wf_sb[:, j, :], wf_d[:, WFA + j, :])

                def wf_j(j):
                    return wfa_sb[:, j, :] if j < WFA else wf_sb[:, j - WFA, :]
                ow_sb = owp.tile([128, DH // 128, E], bf16)
                nc.sync.dma_start(ow_sb[:], o_w3[:])
                for b in range(B):
                    pb = ip_.tile([128, NJ, DH], bf16, name="pb", tag="pb")
                    ub = ip_.tile([128, DH // 128, N], bf16, name="ub",
                                  tag="ub")
                    nc.sync.dma_start(
                        pb[:], psp_d[:, b * 2 * KC:b * 2 * KC + NJ, :])
                    nc.sync.dma_start(
                        ub[:], uT_d[:, :, b * N:(b + 1) * N])
                    gb = gp_.tile([128, DH // 128, N], bf16, name="gb",
                                  tag="gb")
                    # F: tv^T = sum_k P * WF ; gate with u in the evict
                    for m in range(DH // 128):
                        for tch in range(N // 512):
                            ps = ips.tile([128, 512], f32, name="fps",
                                          tag="fps")
                            for j in range(NJ):
                                nc.tensor.matmul(
                                    ps[:], pb[:, j, m * 128:(m + 1) * 128],
                                    wf_j(j)[:, tch * 512:(tch + 1) * 512],
                                    start=(j == 0), stop=(j == NJ - 1))
                            sl = slice(tch * 512, (tch + 1) * 512)
                            nc.vector.tensor_mul(
                                gb[:, m, sl], ps[:], ub[:, m, sl])
                    # H: partial o-projection out_b = g^T @ o_w
                    for mo in range(N // 128):
                        for ech in range(E // 512):
                            ps = ips.tile([128, 512], f32, name="hps",
                                          tag="hps")
                            for kc in range(DH // 128):
                                nc.tensor.matmul(
                                    ps[:], gb[:, kc, mo * 128:(mo + 1) * 128],
                                    ow_sb[:, kc, ech * 512:(ech + 1) * 512],
                                    start=(kc == 0), stop=(kc == DH // 128 - 1))
                            ot = isp.tile([128, 512], f32, name="ot",
                                          tag="ot")
                            nc.scalar.activation(ot[:], ps[:], AFT.Copy)
                            nc.sync.dma_start(
                                out[:, b * (N // 128) + mo,
                                    ech * 512:(ech + 1) * 512], ot[:])

    nc.compile()
    return nc


def _get_nc():
    if "nc" not in _CACHE:
        _CACHE["nc"] = _build()
    return _CACHE["nc"]


def kernel(x, u_w, u_b, v_w, v_b, o_w, o_b,
           pos_w, pos_b, lw0, lb0, lw1, lb1, lw2, lb2, out_w, out_b):
    from concourse.bass_utils import run_bass_kernel_spmd

    wd3, wf3, decay_t = _consts()
    x_flat = np.asarray(x, np.float32).reshape(ROWS, E)
    xTa = np.zeros((KA, ROWS), np.float32)
    xTa[:E] = x_flat.T
    xTa[E] = 1.0
    xTa3 = _t3(xTa, bfl)

    p_aug = np.stack([np.arange(L, dtype=np.float32),
                      np.ones(L, np.float32)])
    pw_aug = np.concatenate([pos_w, pos_b[None, :]], 0).astype(np.float32)
    lbs = np.concatenate(
        [bb.reshape(R // 128, 128).T for bb in (lb0, lb1, lb2)],
        axis=1).astype(np.float32)

    in_maps = []
    for h in range(H):
        sl = slice(h * DH, (h + 1) * DH)
        u_wa = np.zeros((KA, DH), np.float32)
        u_wa[:E] = u_w[:, sl]
        u_wa[E] = u_b[sl]
        v_wa = np.zeros((KA, DH), np.float32)
        v_wa[:E] = v_w[:, sl]
        v_wa[E] = v_b[sl]
        in_maps.append(dict(
            xTa=xTa3, u_wa=_t3(u_wa, bfl), v_wa=_t3(v_wa, bfl),
            o_w3=_t3(np.ascontiguousarray(o_w[sl, :]).astype(np.float32), bfl),
            wd=wd3, wf=wf3,
            p_aug=p_aug, pw_aug=pw_aug,
            lw0=_t3(lw0, bfl), lw1=_t3(lw1, bfl), lw2=_t3(lw2, bfl), lbs=lbs,
            out_w3=_t3(np.ascontiguousarray(out_w[:, sl]), bfl),
            outb=np.ascontiguousarray(out_b[None, sl]).astype(np.float32),
            decay=decay_t,
        ))

    nc = _get_nc()
    res = run_bass_kernel_spmd(nc, in_maps, core_ids=list(range(8)),
                               trace=bool(_CACHE.get("trace")))
    _CACHE["last_res"] = res
    acc = np.zeros((ROWS, E), np.float32)
    for i in range(H):
        acc += _from3(res.results[i]["out"])
    acc += o_b[None, :]
    return acc.reshape(B, N, E)


# revision 8
# speedup vs baseline: 25847.6704x; 1.0445x over previous
"""GTU (gated Toeplitz unit) Bass kernel for 8 TRN2 NeuronCores — v2.

Sharding: tensor-parallel over heads (H=8 -> 1 head/core); host sums the
8 partial o-projections.

vs the fp32 baseline (4.98 ms -> 0.94 ms simulated):
- All matmuls in bf16 (1 PE cycle/row vs 4 for fp32); fp32 PSUM
  accumulation; norm math in the RPE MLP stays fp32.
- Kernel lags truncated at L=512 (decay gamma^512 ~ 5.8e-3), shrinking
  the circular conv from 4096 to M2=2560 points and the RPE MLP to the
  512 positions that survive the decay.
- One SBUF-resident DFT matrix per phase, loaded once (not per batch)
  and prefetched on the second (Activation) DMA queue; forward spectra,
  complex multiply and gate all stay on-chip; only the P spectrum
  round-trips DRAM between the two DFT phases. The Nyquist bin lands on a chunk
  boundary, so the all-zero Nyquist-sine chunk is skipped everywhere.
- u/v projections fused into one pass streaming x^T tiles used as both
  moving (uT) and stationary (v) matmul operands, overlapped with the
  serial MLP chains to keep the PE fed.
"""

import numpy as np
import ml_dtypes

B, N, E = 4, 2048, 1024
H = 8
D1 = 3 * E
DH = D1 // H            # 384
R = 512
GAMMA = 0.99
EPS = 1e-8
L = 512                 # truncated kernel lags (4*128)
LC = L // 128           # 4
M2 = 2560               # circular conv length >= N + L - 1
KH = M2 // 2 + 1        # 1281 rfft bins
KC = 11                 # freq chunks of 128 (pad 1281 -> 1408)
KP = KC * 128           # 1408
ROWS = B * N            # 8192
KA = 1152               # augmented contraction for x (bias row), 9*128

_CACHE = {}

bfl = ml_dtypes.bfloat16


def _t3(a, dtype=np.float32):
    """(M, N) -> (128, M/128, N) partition-tiled layout."""
    m, n = a.shape
    assert m % 128 == 0
    return np.ascontiguousarray(
        a.reshape(m // 128, 128, n).transpose(1, 0, 2)).astype(dtype)


def _from3(a):
    p, m, n = a.shape
    return np.ascontiguousarray(
        np.asarray(a, np.float32).transpose(1, 0, 2)).reshape(m * 128, n)


def _consts():
    if "dft" in _CACHE:
        return _CACHE["dft"]
    t = np.arange(N, dtype=np.float64)[:, None]
    k = np.arange(KP, dtype=np.float64)[None, :]
    mask = (k <= (KH - 1)).astype(np.float64)
    ang = 2.0 * np.pi * t * k / M2
    cr = np.cos(ang) * mask
    ci = -np.sin(ang) * mask
    wd = np.concatenate([cr, ci], axis=1)                 # (2048, 3072)

    kk = np.arange(KP, dtype=np.float64)[:, None]
    tt = np.arange(N, dtype=np.float64)[None, :]
    w = np.where((kk == 0) | (kk == M2 // 2), 1.0, 2.0) * (kk <= (KH - 1)) / M2
    ang2 = 2.0 * np.pi * kk * tt / M2
    icos = w * np.cos(ang2)
    isin = (-w * np.sin(ang2))[:KP - 128]  # last sine chunk is all zero
    wf = np.concatenate([icos, isin], axis=0)             # (2688, 2048)

    decay = GAMMA ** np.arange(L, dtype=np.float64)       # lag 0 -> 1.0
    decay_t = decay.reshape(LC, 128).T                    # (128, 6)
    _CACHE["dft"] = (_t3(wd, bfl), _t3(wf, bfl), decay_t.astype(np.float32))
    return _CACHE["dft"]


def _build():
    import concourse.bass as bass
    import concourse.mybir as mybir
    import concourse.tile as tile
    from concourse import bacc

    AFT = mybir.ActivationFunctionType
    ALU = mybir.AluOpType
    f32 = mybir.dt.float32
    f32r = mybir.dt.float32r
    bf16 = mybir.dt.bfloat16

    nc = bacc.Bacc(None, target_bir_lowering=False, debug=False, num_devices=8)

    def din(name, shape, dt=f32):
        return nc.dram_tensor(name, list(shape), dt, kind="ExternalInput")

    def dint(name, shape, dt=bf16):
        return nc.dram_tensor(name, list(shape), dt)

    # inputs
    xTa = din("xTa", (128, KA // 128, ROWS), bf16)
    u_wa = din("u_wa", (128, KA // 128, DH), bf16)
    u_b3 = din("u_b3", (128, DH // 128))
    v_wa = din("v_wa", (128, KA // 128, DH), bf16)
    o_w3 = din("o_w3", (128, DH // 128, E), bf16)
    wd_d = din("wd", (128, N // 128, 2 * KP), bf16)
    wf_d = din("wf", (128, 2 * KC - 1, N), bf16)
    p_aug = din("p_aug", (2, L))
    pw_aug = din("pw_aug", (2, R))
    lws = [din(f"lw{i}", (128, R // 128, R), bf16) for i in range(3)]
    lbs = din("lbs", (128, 3 * (R // 128)))
    out_w3 = din("out_w3", (128, R // 128, DH), bf16)
    outb = din("outb", (1, DH))
    decay = din("decay", (128, LC))
    out = nc.dram_tensor("out", [128, ROWS // 128, E], f32,
                         kind="ExternalOutput")

    # dram temps (bf16)
    uT_d = dint("uT_d", (128, DH // 128, ROWS))
    v_d = dint("v_d", (128, ROWS // 128, DH))
    psp_d = dint("psp_d", (128, B * 2 * KC, DH))

    FG = R // 128             # 4 feature groups
    PC = 256                  # MLP position-chunk width
    NCH = L // PC             # 2 position chunks in the (truncated) MLP

    with tile.TileContext(nc) as tc, nc.allow_low_precision(
            reason="bf16 pipeline validated against fp32 reference"):
        with tc.tile_pool(name="persist", bufs=1) as pp:
            acoef = pp.tile([128, LC, DH], bf16)   # truncated decayed coefs

            # wd is loaded up front: its pool sits above the phase-0/1
            # working set, so the 12.6MB DMA overlaps the MLP + u/v phase
            # instead of stalling the DFT phase behind it.
            uw_sb = pp.tile([128, KA // 128, DH], bf16)
            vw_sb = pp.tile([128, KA // 128, DH], bf16)
            ub_sb = pp.tile([128, DH // 128], f32)
            nc.sync.dma_start(ub_sb[:], u_b3[:])
            WFA = 4
            wfa_sb = pp.tile([128, WFA, N], bf16)
            nc.scalar.dma_start(uw_sb[:], u_wa[:])
            nc.scalar.dma_start(vw_sb[:], v_wa[:])
            wd_pool = tc.tile_pool(name="wd", bufs=1)
            wdp = wd_pool.__enter__()
            wd_sb = wdp.tile([128, N // 128, 2 * KP], bf16)
            for kc in range(N // 128):
                nc.scalar.dma_start(wd_sb[:, kc, :], wd_d[:, kc, :])

            # ------- RPE MLP + u/v projections (concurrent engines) -----
            # The MLP's serial norm->matmul chains leave the PE idle; the
            # u/v GEMMs stream through the same window and fill it.
            with (tc.tile_pool(name="mlp", bufs=1) as mp,
                  tc.tile_pool(name="mlp_ps", bufs=1, space="PSUM") as mps,
                  tc.tile_pool(name="uvx", bufs=3) as xp,
                  tc.tile_pool(name="uvs", bufs=4) as sp,
                  tc.tile_pool(name="uv_ps", bufs=2, space="PSUM") as ups):
                # Engines run their streams in order, so the serial MLP
                # chain must be INTERLEAVED with the u/v GEMM groups at
                # emission time or it just runs after them. uv_feed(k)
                # emits the next k groups; it is called between MLP
                # pipeline stages.
                uv_pending = list(range(ROWS // 512))

                def uv_group(grp):
                    xt = xp.tile([128, KA // 128, 512], bf16, name="xt",
                                 tag="xt")
                    nc.sync.dma_start(
                        xt[:], xTa[:, :, grp * 512:(grp + 1) * 512])
                    # uT tile: out[M=DH, N=512 rows]
                    for m in range(DH // 128):
                        ps = ups.tile([128, 512], f32, name="bps", tag="bps")
                        for kc in range(KA // 128 - 1):  # bias via evict
                            nc.tensor.matmul(
                                ps[:], uw_sb[:, kc, m * 128:(m + 1) * 128],
                                xt[:, kc, :], start=(kc == 0),
                                stop=(kc == KA // 128 - 2))
                        ut = sp.tile([128, 512], bf16, name="ut", tag="ut")
                        nc.scalar.activation(ut[:], ps[:], AFT.Silu,
                                             bias=ub_sb[:, m:m + 1])
                        nc.sync.dma_start(
                            uT_d[:, m, grp * 512:(grp + 1) * 512], ut[:])
                    # v tiles: out[M=128 rows, N=DH]
                    for rs in range(4):
                        ps = ups.tile([128, DH], f32, name="cps", tag="cps")
                        for kc in range(KA // 128):
                            nc.tensor.matmul(
                                ps[:], xt[:, kc, rs * 128:(rs + 1) * 128],
                                vw_sb[:, kc, :], start=(kc == 0),
                                stop=(kc == KA // 128 - 1))
                        vt = sp.tile([128, DH], bf16, name="vt", tag="vt")
                        nc.scalar.activation(vt[:], ps[:], AFT.Silu)
                        nc.sync.dma_start(v_d[:, grp * 4 + rs, :], vt[:])

                def uv_feed(k):
                    for _ in range(min(k, len(uv_pending))):
                        uv_group(uv_pending.pop(0))

                uv_feed(1)
                ones_col = mp.tile([128, 1], bf16)     # K=128 -> M=1 reducer
                nc.vector.memset(ones_col[:], 1.0)
                one_row = mp.tile([1, 128], bf16)      # K=1 -> 128-part bcast
                nc.vector.memset(one_row[:], 1.0)
                one_rowf = mp.tile([1, 128], f32)
                nc.vector.memset(one_rowf[:], 1.0)
                c_sc = mp.tile([1, 1], f32)
                nc.vector.memset(c_sc[:], float(R ** -0.5))
                eps_sc = mp.tile([1, 1], f32)
                nc.vector.memset(eps_sc[:], EPS)

                pa_sb = mp.tile([2, L], f32)
                pw_sb = mp.tile([2, R], f32)
                lb_sb = mp.tile([128, 3 * FG], f32)
                nc.sync.dma_start(pa_sb[:], p_aug[:])
                nc.sync.dma_start(pw_sb[:], pw_aug[:])
                nc.sync.dma_start(lb_sb[:], lbs[:])

                # MLP runs only on the L kept lags; fp32 h, bf16 matmuls
                h = [mp.tile([128, L], f32, name=f"h{g}", tag=f"h{g}")
                     for g in range(FG)]
                # h0 = pos_idx @ pos_w + pos_b (K=2, fp32), feature-major
                for g in range(FG):
                    for nch in range(NCH):
                        ps = mps.tile([128, PC], f32, name="mmps", tag="mm")
                        nc.tensor.matmul(
                            ps[:], pw_sb[:, g * 128:(g + 1) * 128],
                            pa_sb[:, nch * PC:(nch + 1) * PC],
                            start=True, stop=True)
                        nc.vector.tensor_copy(
                            h[g][:, nch * PC:(nch + 1) * PC], ps[:])

                def srms_relu(h_in, phi_out):
                    # s[t] = sum_f h^2 ; factor = 1/(sqrt(s)/sqrt(R) + eps)
                    sq = [mp.tile([128, L], bf16, name=f"sq{g}", tag=f"sq{g}")
                          for g in range(FG)]
                    for g in range(FG):
                        nc.vector.tensor_mul(sq[g][:], h_in[g][:], h_in[g][:])
                    facb = mp.tile([1, L], bf16, name="facb", tag="facb")
                    fac = mp.tile([1, L], f32, name="fac", tag="fac")
                    for nch in range(NCH):
                        ps1 = mps.tile([1, PC], f32, name="redps", tag="red")
                        for g in range(FG):
                            nc.tensor.matmul(
                                ps1[:], ones_col[:],
                                sq[g][:, nch * PC:(nch + 1) * PC],
                                start=(g == 0), stop=(g == FG - 1))
                        sl = fac[:, nch * PC:(nch + 1) * PC]
                        nc.scalar.activation(sl, ps1[:], AFT.Sqrt)
                        nc.vector.tensor_scalar(
                            sl, sl, c_sc[:], eps_sc[:], ALU.mult, ALU.add)
                        nc.vector.reciprocal(
                            facb[:, nch * PC:(nch + 1) * PC], sl)
                    fb = mp.tile([128, L], f32, name="fb", tag="fb")
                    for nch in range(NCH):
                        psb = mps.tile([128, PC], f32, name="bcps", tag="bc")
                        nc.tensor.matmul(
                            psb[:], one_row[:],
                            facb[:, nch * PC:(nch + 1) * PC],
                            start=True, stop=True)
                        nc.vector.tensor_copy(
                            fb[:, nch * PC:(nch + 1) * PC], psb[:])
                    for g in range(FG):
                        nc.vector.tensor_mul(phi_out[g][:], h_in[g][:], fb[:])
                        nc.scalar.activation(
                            phi_out[g][:], phi_out[g][:], AFT.Relu)

                phi = [mp.tile([128, L], bf16, name=f"phi{g}", tag=f"phi{g}")
                       for g in range(FG)]
                uv_feed(1)
                srms_relu(h, phi)
                uv_feed(2)

                lw_sb = mp.tile([128, FG, R], bf16)
                for li in range(3):
                    nc.sync.dma_start(lw_sb[:], lws[li][:])
                    for g in range(FG):
                        for nch in range(NCH):
                            ps = mps.tile([128, PC], f32, name="mmps",
                                          tag="mm")
                            for kk in range(FG):
                                nc.tensor.matmul(
                                    ps[:],
                                    lw_sb[:, kk, g * 128:(g + 1) * 128],
                                    phi[kk][:, nch * PC:(nch + 1) * PC],
                                    start=(kk == 0), stop=(kk == FG - 1))
                            sl = h[g][:, nch * PC:(nch + 1) * PC]
                            nc.vector.tensor_scalar(
                                sl, ps[:],
                                lb_sb[:, li * FG + g:li * FG + g + 1],
                                None, ALU.add)
                    uv_feed(2)
                    srms_relu(h, phi)
                    uv_feed(2)

                # coefs (t-major, lags < L only) = phi.T @ out_w
                ow_sb = mp.tile([128, FG, DH], bf16)
                ob_sb = mp.tile([1, DH], f32)
                dec_sb = mp.tile([128, LC], f32)
                nc.sync.dma_start(ow_sb[:], out_w3[:])
                nc.sync.dma_start(ob_sb[:], outb[:])
                nc.sync.dma_start(dec_sb[:], decay[:])
                obb = mp.tile([128, DH], f32)
                psb = mps.tile([128, DH], f32, name="bc2ps", tag="bc")
                nc.tensor.matmul(psb[:], one_rowf[:], ob_sb[:],
                                 start=True, stop=True)
                nc.vector.tensor_copy(obb[:], psb[:])
                uv_feed(1)
                for m in range(LC):
                    uv_feed(1)
                    ps = mps.tile([128, DH], f32, name="mm2ps", tag="mm")
                    for kk in range(FG):
                        nc.tensor.matmul(
                            ps[:], phi[kk][:, m * 128:(m + 1) * 128],
                            ow_sb[:, kk, :], start=(kk == 0),
                            stop=(kk == FG - 1))
                    ac = mp.tile([128, DH], f32, name="ac", tag="ac")
                    nc.vector.tensor_add(ac[:], ps[:], obb[:])
                    nc.vector.tensor_scalar(
                        acoef[:, m, :], ac[:], dec_sb[:, m:m + 1],
                        None, ALU.mult)
                uv_feed(ROWS // 512)

            # wf head-piece: loads during phase 2 (persist pool, so no
            # dependence on wd's release), letting the inverse phase
            # start before the full wf load completes.
            for j in range(WFA):
                nc.scalar.dma_start(wfa_sb[:, j, :], wf_d[:, j, :])

            # ---------------- forward DFTs + complex multiply -----------
            with (tc.tile_pool(name="fwd", bufs=1) as fp_,
                  tc.tile_pool(name="fwdx", bufs=2) as fpx,
                  tc.tile_pool(name="fwdv", bufs=2) as fpv,
                  tc.tile_pool(name="fwd2", bufs=4) as fp2,
                  tc.tile_pool(name="fwd_ps", bufs=4, space="PSUM") as fps):
                asp = fp_.tile([128, 2 * KC, DH], bf16)    # kernel spectrum
                # A: Ar/Ai m-tiles, contraction over L lags only
                for m in range(2 * KC - 1):
                    ps = fps.tile([128, DH], f32, name="aps", tag="aps")
                    for kc in range(LC):
                        nc.tensor.matmul(
                            ps[:], wd_sb[:, kc, m * 128:(m + 1) * 128],
                            acoef[:, kc, :], start=(kc == 0),
                            stop=(kc == LC - 1))
                    nc.scalar.activation(asp[:, m, :], ps[:], AFT.Copy)
                for b in range(B):
                    vb = fpv.tile([128, N // 128, DH], bf16, name="vb",
                                  tag="vb")
                    nc.sync.dma_start(
                        vb[:], v_d[:, b * (N // 128):(b + 1) * (N // 128), :])
                    xsp = fpx.tile([128, 2 * KC, DH], bf16, name="xsp",
                                   tag="xsp")
                    # D: X = DFT(v_b); interleave re/im pairs for E
                    for j in range(KC):
                        ms = (j,) if j == KC - 1 else (j, KC + j)
                        for m in ms:
                            ps = fps.tile([128, DH], f32, name="dps",
                                          tag="dps")
                            for kc in range(N // 128):
                                nc.tensor.matmul(
                                    ps[:],
                                    wd_sb[:, kc, m * 128:(m + 1) * 128],
                                    vb[:, kc, :], start=(kc == 0),
                                    stop=(kc == N // 128 - 1))
                            nc.scalar.activation(xsp[:, m, :], ps[:],
                                                 AFT.Copy)
                        # E: P = A * X (complex), in place over xsp
                        xr, xi = xsp[:, j, :], xsp[:, KC + j, :]
                        ar, ai = asp[:, j, :], asp[:, KC + j, :]
                        if j == KC - 1:
                            nc.vector.tensor_mul(xr, ar, xr)
                            nc.sync.dma_start(
                                psp_d[:, b * 2 * KC + j, :], xr)
                            continue
                        t1 = fp2.tile([128, DH], bf16, name="t1", tag="t1")
                        t2 = fp2.tile([128, DH], bf16, name="t2", tag="t2")
                        t3 = fp2.tile([128, DH], bf16, name="t3", tag="t3")
                        nc.vector.tensor_mul(t1[:], ar, xr)
                        nc.vector.tensor_mul(t2[:], ai, xi)
                        nc.vector.tensor_mul(t3[:], ar, xi)
                        nc.vector.tensor_mul(xi, ai, xr)
                        nc.vector.tensor_sub(xr, t1[:], t2[:])
                        nc.vector.tensor_add(xi, t3[:], xi)
                        nc.sync.dma_start(psp_d[:, b * 2 * KC + j, :], xr)
                        nc.sync.dma_start(
                            psp_d[:, b * 2 * KC + KC + j, :], xi)
            wd_pool.__exit__(None, None, None)


            # ---------------- inverse DFT + gate + o-projection ---------
            with (tc.tile_pool(name="wf", bufs=1) as wfp,
                  tc.tile_pool(name="inv", bufs=2) as ip_,
                  tc.tile_pool(name="invg", bufs=2) as gp_,
                  tc.tile_pool(name="invw", bufs=1) as owp,
                  tc.tile_pool(name="invs", bufs=4) as isp,
                  tc.tile_pool(name="inv_ps", bufs=4, space="PSUM") as ips):
                NJ = 2 * KC - 1
                wf_sb = wfp.tile([128, NJ - WFA, N], bf16)
                for j in range(NJ - WFA):  # tail: F pipelines with the load
                    nc.scalar.dma_start(wf_sb[:, j, :], wf_d[:, WFA + j, :])

                def wf_j(j):
                    return wfa_sb[:, j, :] if j < WFA else wf_sb[:, j - WFA, :]
                ow_sb = owp.tile([128, DH // 128, E], bf16)
                nc.sync.dma_start(ow_sb[:], o_w3[:])
                for b in range(B):
                    pb = ip_.tile([128, NJ, DH], bf16, name="pb", tag="pb")
                    ub = ip_.tile([128, DH // 128, N], bf16, name="ub",
                                  tag="ub")
                    nc.sync.dma_start(
                        pb[:], psp_d[:, b * 2 * KC:b * 2 * KC + NJ, :])
                    nc.sync.dma_start(
                        ub[:], uT_d[:, :, b * N:(b + 1) * N])
                    gb = gp_.tile([128, DH // 128, N], bf16, name="gb",
                                  tag="gb")
                    # F: tv^T = sum_k P * WF ; gate with u in the evict
                    for m in range(DH // 128):
                        for tch in range(N // 512):
                            ps = ips.tile([128, 512], f32, name="fps",
                                          tag="fps")
                            for j in range(NJ):
                                nc.tensor.matmul(
                                    ps[:], pb[:, j, m * 128:(m + 1) * 128],
                                    wf_j(j)[:, tch * 512:(tch + 1) * 512],
                                    start=(j == 0), stop=(j == NJ - 1))
                            sl = slice(tch * 512, (tch + 1) * 512)
                            nc.vector.tensor_mul(
                                gb[:, m, sl], ps[:], ub[:, m, sl])
                    # H: partial o-projection out_b = g^T @ o_w
                    for mo in range(N // 128):
                        for ech in range(E // 512):
                            ps = ips.tile([128, 512], f32, name="hps",
                                          tag="hps")
                            for kc in range(DH // 128):
                                nc.tensor.matmul(
                                    ps[:], gb[:, kc, mo * 128:(mo + 1) * 128],
                                    ow_sb[:, kc, ech * 512:(ech + 1) * 512],
                                    start=(kc == 0), stop=(kc == DH // 128 - 1))
                            ot = isp.tile([128, 512], f32, name="ot",
                                          tag="ot")
                            nc.scalar.activation(ot[:], ps[:], AFT.Copy)
                            nc.sync.dma_start(
                                out[:, b * (N // 128) + mo,
                                    ech * 512:(ech + 1) * 512], ot[:])

    nc.compile()
    return nc


def _get_nc():
    if "nc" not in _CACHE:
        _CACHE["nc"] = _build()
    return _CACHE["nc"]


def kernel(x, u_w, u_b, v_w, v_b, o_w, o_b,
           pos_w, pos_b, lw0, lb0, lw1, lb1, lw2, lb2, out_w, out_b):
    from concourse.bass_utils import run_bass_kernel_spmd

    wd3, wf3, decay_t = _consts()
    x_flat = np.asarray(x, np.float32).reshape(ROWS, E)
    xTa = np.zeros((KA, ROWS), np.float32)
    xTa[:E] = x_flat.T
    xTa[E] = 1.0
    xTa3 = _t3(xTa, bfl)

    p_aug = np.stack([np.arange(L, dtype=np.float32),
                      np.ones(L, np.float32)])
    pw_aug = np.concatenate([pos_w, pos_b[None, :]], 0).astype(np.float32)
    lbs = np.concatenate(
        [bb.reshape(R // 128, 128).T for bb in (lb0, lb1, lb2)],
        axis=1).astype(np.float32)

    in_maps = []
    for h in range(H):
        sl = slice(h * DH, (h + 1) * DH)
        u_wa = np.zeros((KA, DH), np.float32)
        u_wa[:E] = u_w[:, sl]
        u_wa[E] = u_b[sl]
        v_wa = np.zeros((KA, DH), np.float32)
        v_wa[:E] = v_w[:, sl]
        v_wa[E] = v_b[sl]
        in_maps.append(dict(
            xTa=xTa3, u_wa=_t3(u_wa, bfl), v_wa=_t3(v_wa, bfl),
            u_b3=np.ascontiguousarray(
                u_b[sl].reshape(DH // 128, 128).T).astype(np.float32),
            o_w3=_t3(np.ascontiguousarray(o_w[sl, :]).astype(np.float32), bfl),
            wd=wd3, wf=wf3,
            p_aug=p_aug, pw_aug=pw_aug,
            lw0=_t3(lw0, bfl), lw1=_t3(lw1, bfl), lw2=_t3(lw2, bfl), lbs=lbs,
            out_w3=_t3(np.ascontiguousarray(out_w[:, sl]), bfl),
            outb=np.ascontiguousarray(out_b[None, sl]).astype(np.float32),
            decay=decay_t,
        ))

    nc = _get_nc()
    res = run_bass_kernel_spmd(nc, in_maps, core_ids=list(range(8)),
                               trace=bool(_CACHE.get("trace")))
    _CACHE["last_res"] = res
    acc = np.zeros((ROWS, E), np.float32)
    for i in range(H):
        acc += _from3(res.results[i]["out"])
    acc += o_b[None, :]
    return acc.reshape(B, N, E)


# revision 9
# speedup vs baseline: 26720.7325x; 1.0338x over previous
"""GTU (gated Toeplitz unit) Bass kernel for 8 TRN2 NeuronCores — v2.

Sharding: tensor-parallel over heads (H=8 -> 1 head/core); host sums the
8 partial o-projections.

vs the fp32 baseline (4.98 ms -> 0.94 ms simulated):
- All matmuls in bf16 (1 PE cycle/row vs 4 for fp32); fp32 PSUM
  accumulation; norm math in the RPE MLP stays fp32.
- Kernel lags truncated at L=512 (decay gamma^512 ~ 5.8e-3), shrinking
  the circular conv from 4096 to M2=2560 points and the RPE MLP to the
  512 positions that survive the decay.
- One SBUF-resident DFT matrix per phase, loaded once (not per batch)
  and prefetched on the second (Activation) DMA queue; forward spectra,
  complex multiply and gate all stay on-chip; only the P spectrum
  round-trips DRAM between the two DFT phases. The Nyquist bin lands on a chunk
  boundary, so the all-zero Nyquist-sine chunk is skipped everywhere.
- u/v projections fused into one pass streaming x^T tiles used as both
  moving (uT) and stationary (v) matmul operands, overlapped with the
  serial MLP chains to keep the PE fed.
"""

import numpy as np
import ml_dtypes

B, N, E = 4, 2048, 1024
H = 8
D1 = 3 * E
DH = D1 // H            # 384
R = 512
GAMMA = 0.99
EPS = 1e-8
L = 512                 # truncated kernel lags (4*128)
LC = L // 128           # 4
M2 = 2560               # circular conv length >= N + L - 1
KH = M2 // 2 + 1        # 1281 rfft bins
KC = 11                 # freq chunks of 128 (pad 1281 -> 1408)
KP = KC * 128           # 1408
ROWS = B * N            # 8192
KA = 1152               # augmented contraction for x (bias row), 9*128

_CACHE = {}

bfl = ml_dtypes.bfloat16


def _t3(a, dtype=np.float32):
    """(M, N) -> (128, M/128, N) partition-tiled layout."""
    m, n = a.shape
    assert m % 128 == 0
    return np.ascontiguousarray(
        a.reshape(m // 128, 128, n).transpose(1, 0, 2)).astype(dtype)


def _from3(a):
    p, m, n = a.shape
    return np.ascontiguousarray(
        np.asarray(a, np.float32).transpose(1, 0, 2)).reshape(m * 128, n)


def _consts():
    if "dft" in _CACHE:
        return _CACHE["dft"]
    t = np.arange(N, dtype=np.float64)[:, None]
    k = np.arange(KP, dtype=np.float64)[None, :]
    mask = (k <= (KH - 1)).astype(np.float64)
    ang = 2.0 * np.pi * t * k / M2
    cr = np.cos(ang) * mask
    ci = -np.sin(ang) * mask
    wd = np.concatenate([cr, ci], axis=1)                 # (2048, 3072)

    kk = np.arange(KP, dtype=np.float64)[:, None]
    tt = np.arange(N, dtype=np.float64)[None, :]
    w = np.where((kk == 0) | (kk == M2 // 2), 1.0, 2.0) * (kk <= (KH - 1)) / M2
    ang2 = 2.0 * np.pi * kk * tt / M2
    icos = w * np.cos(ang2)
    isin = (-w * np.sin(ang2))[:KP - 128]  # last sine chunk is all zero
    wf = np.concatenate([icos, isin], axis=0)             # (2688, 2048)

    decay = GAMMA ** np.arange(L, dtype=np.float64)       # lag 0 -> 1.0
    decay_t = decay.reshape(LC, 128).T                    # (128, 6)
    _CACHE["dft"] = (_t3(wd, bfl), _t3(wf, bfl), decay_t.astype(np.float32))
    return _CACHE["dft"]


def _build():
    import concourse.bass as bass
    import concourse.mybir as mybir
    import concourse.tile as tile
    from concourse import bacc

    AFT = mybir.ActivationFunctionType
    ALU = mybir.AluOpType
    f32 = mybir.dt.float32
    f32r = mybir.dt.float32r
    bf16 = mybir.dt.bfloat16

    nc = bacc.Bacc(None, target_bir_lowering=False, debug=False, num_devices=8)

    def din(name, shape, dt=f32):
        return nc.dram_tensor(name, list(shape), dt, kind="ExternalInput")

    def dint(name, shape, dt=bf16):
        return nc.dram_tensor(name, list(shape), dt)

    # inputs
    xTa = din("xTa", (128, KA // 128, ROWS), bf16)
    u_wa = din("u_wa", (128, KA // 128, DH), bf16)
    u_b3 = din("u_b3", (128, DH // 128))
    v_wa = din("v_wa", (128, KA // 128, DH), bf16)
    o_w3 = din("o_w3", (128, DH // 128, E), bf16)
    wd_d = din("wd", (128, N // 128, 2 * KP), bf16)
    wf_d = din("wf", (128, 2 * KC - 1, N), bf16)
    p_aug = din("p_aug", (2, L))
    pw_aug = din("pw_aug", (2, R))
    lws = [din(f"lw{i}", (128, R // 128, R), bf16) for i in range(3)]
    lbs = din("lbs", (128, 3 * (R // 128)))
    out_w3 = din("out_w3", (128, R // 128, DH), bf16)
    outb = din("outb", (1, DH))
    decay = din("decay", (128, LC))
    out = nc.dram_tensor("out", [128, ROWS // 128, E], f32,
                         kind="ExternalOutput")

    # dram temps (bf16)
    uT_d = dint("uT_d", (128, DH // 128, ROWS))
    v_d = dint("v_d", (128, ROWS // 128, DH))
    psp_d = dint("psp_d", (128, B * 2 * KC, DH))

    FG = R // 128             # 4 feature groups
    PC = 256                  # MLP position-chunk width
    NCH = L // PC             # 2 position chunks in the (truncated) MLP

    with tile.TileContext(nc) as tc, nc.allow_low_precision(
            reason="bf16 pipeline validated against fp32 reference"):
        with tc.tile_pool(name="persist", bufs=1) as pp:
            acoef = pp.tile([128, LC, DH], bf16)   # truncated decayed coefs

            # wd is loaded up front: its pool sits above the phase-0/1
            # working set, so the 12.6MB DMA overlaps the MLP + u/v phase
            # instead of stalling the DFT phase behind it.
            uw_sb = pp.tile([128, KA // 128, DH], bf16)
            vw_sb = pp.tile([128, KA // 128, DH], bf16)
            ub_sb = pp.tile([128, DH // 128], f32)
            nc.sync.dma_start(ub_sb[:], u_b3[:])
            WFA = 4
            wfa_sb = pp.tile([128, WFA, N], bf16)
            nc.scalar.dma_start(uw_sb[:], u_wa[:])
            nc.scalar.dma_start(vw_sb[:], v_wa[:])
            wd_pool = tc.tile_pool(name="wd", bufs=1)
            wdp = wd_pool.__enter__()
            wd_sb = wdp.tile([128, N // 128, 2 * KP], bf16)
            # wd chunks are fed one-per-uv-group through the SP queue: on
            # the Act queue they would sit ahead of the MLP's activations
            # in the Act stream and stall its first norm by ~14us.

            # ------- RPE MLP + u/v projections (concurrent engines) -----
            # The MLP's serial norm->matmul chains leave the PE idle; the
            # u/v GEMMs stream through the same window and fill it.
            with (tc.tile_pool(name="mlp", bufs=1) as mp,
                  tc.tile_pool(name="mlp_ps", bufs=1, space="PSUM") as mps,
                  tc.tile_pool(name="uvx", bufs=3) as xp,
                  tc.tile_pool(name="uvs", bufs=4) as sp,
                  tc.tile_pool(name="uv_ps", bufs=2, space="PSUM") as ups):
                # Engines run their streams in order, so the serial MLP
                # chain must be INTERLEAVED with the u/v GEMM groups at
                # emission time or it just runs after them. uv_feed(k)
                # emits the next k groups; it is called between MLP
                # pipeline stages.
                uv_pending = list(range(ROWS // 512))

                def uv_group(grp):
                    xt = xp.tile([128, KA // 128, 512], bf16, name="xt",
                                 tag="xt")
                    nc.sync.dma_start(
                        xt[:], xTa[:, :, grp * 512:(grp + 1) * 512])
                    nc.sync.dma_start(wd_sb[:, grp, :], wd_d[:, grp, :])
                    # uT tile: out[M=DH, N=512 rows]
                    for m in range(DH // 128):
                        ps = ups.tile([128, 512], f32, name="bps", tag="bps")
                        for kc in range(KA // 128 - 1):  # bias via evict
                            nc.tensor.matmul(
                                ps[:], uw_sb[:, kc, m * 128:(m + 1) * 128],
                                xt[:, kc, :], start=(kc == 0),
                                stop=(kc == KA // 128 - 2))
                        ut = sp.tile([128, 512], bf16, name="ut", tag="ut")
                        nc.scalar.activation(ut[:], ps[:], AFT.Silu,
                                             bias=ub_sb[:, m:m + 1])
                        nc.sync.dma_start(
                            uT_d[:, m, grp * 512:(grp + 1) * 512], ut[:])
                    # v tiles: out[M=128 rows, N=DH]
                    for rs in range(4):
                        ps = ups.tile([128, DH], f32, name="cps", tag="cps")
                        for kc in range(KA // 128):
                            nc.tensor.matmul(
                                ps[:], xt[:, kc, rs * 128:(rs + 1) * 128],
                                vw_sb[:, kc, :], start=(kc == 0),
                                stop=(kc == KA // 128 - 1))
                        vt = sp.tile([128, DH], bf16, name="vt", tag="vt")
                        nc.scalar.activation(vt[:], ps[:], AFT.Silu)
                        nc.sync.dma_start(v_d[:, grp * 4 + rs, :], vt[:])

                def uv_feed(k):
                    for _ in range(min(k, len(uv_pending))):
                        uv_group(uv_pending.pop(0))

                uv_feed(1)
                ones_col = mp.tile([128, 1], bf16)     # K=128 -> M=1 reducer
                nc.vector.memset(ones_col[:], 1.0)
                one_row = mp.tile([1, 128], bf16)      # K=1 -> 128-part bcast
                nc.vector.memset(one_row[:], 1.0)
                one_rowf = mp.tile([1, 128], f32)
                nc.vector.memset(one_rowf[:], 1.0)
                c_sc = mp.tile([1, 1], f32)
                nc.vector.memset(c_sc[:], float(R ** -0.5))
                eps_sc = mp.tile([1, 1], f32)
                nc.vector.memset(eps_sc[:], EPS)

                pa_sb = mp.tile([2, L], f32)
                pw_sb = mp.tile([2, R], f32)
                lb_sb = mp.tile([128, 3 * FG], f32)
                nc.sync.dma_start(pa_sb[:], p_aug[:])
                nc.sync.dma_start(pw_sb[:], pw_aug[:])
                nc.sync.dma_start(lb_sb[:], lbs[:])

                # MLP runs only on the L kept lags; fp32 h, bf16 matmuls
                h = [mp.tile([128, L], f32, name=f"h{g}", tag=f"h{g}")
                     for g in range(FG)]
                # h0 = pos_idx @ pos_w + pos_b (K=2, fp32), feature-major
                for g in range(FG):
                    for nch in range(NCH):
                        ps = mps.tile([128, PC], f32, name="mmps", tag="mm")
                        nc.tensor.matmul(
                            ps[:], pw_sb[:, g * 128:(g + 1) * 128],
                            pa_sb[:, nch * PC:(nch + 1) * PC],
                            start=True, stop=True)
                        nc.vector.tensor_copy(
                            h[g][:, nch * PC:(nch + 1) * PC], ps[:])

                def srms_relu(h_in, phi_out):
                    # s[t] = sum_f h^2 ; factor = 1/(sqrt(s)/sqrt(R) + eps)
                    sq = [mp.tile([128, L], bf16, name=f"sq{g}", tag=f"sq{g}")
                          for g in range(FG)]
                    for g in range(FG):
                        nc.vector.tensor_mul(sq[g][:], h_in[g][:], h_in[g][:])
                    facb = mp.tile([1, L], bf16, name="facb", tag="facb")
                    fac = mp.tile([1, L], f32, name="fac", tag="fac")
                    for nch in range(NCH):
                        ps1 = mps.tile([1, PC], f32, name="redps", tag="red")
                        for g in range(FG):
                            nc.tensor.matmul(
                                ps1[:], ones_col[:],
                                sq[g][:, nch * PC:(nch + 1) * PC],
                                start=(g == 0), stop=(g == FG - 1))
                        sl = fac[:, nch * PC:(nch + 1) * PC]
                        nc.scalar.activation(sl, ps1[:], AFT.Sqrt)
                        nc.vector.tensor_scalar(
                            sl, sl, c_sc[:], eps_sc[:], ALU.mult, ALU.add)
                        nc.vector.reciprocal(
                            facb[:, nch * PC:(nch + 1) * PC], sl)
                    fb = mp.tile([128, L], f32, name="fb", tag="fb")
                    for nch in range(NCH):
                        psb = mps.tile([128, PC], f32, name="bcps", tag="bc")
                        nc.tensor.matmul(
                            psb[:], one_row[:],
                            facb[:, nch * PC:(nch + 1) * PC],
                            start=True, stop=True)
                        nc.vector.tensor_copy(
                            fb[:, nch * PC:(nch + 1) * PC], psb[:])
                    for g in range(FG):
                        nc.vector.tensor_mul(phi_out[g][:], h_in[g][:], fb[:])
                        nc.scalar.activation(
                            phi_out[g][:], phi_out[g][:], AFT.Relu)

                phi = [mp.tile([128, L], bf16, name=f"phi{g}", tag=f"phi{g}")
                       for g in range(FG)]
                uv_feed(1)
                srms_relu(h, phi)
                uv_feed(2)

                lw_sb = mp.tile([128, FG, R], bf16)
                for li in range(3):
                    nc.sync.dma_start(lw_sb[:], lws[li][:])
                    for g in range(FG):
                        for nch in range(NCH):
                            ps = mps.tile([128, PC], f32, name="mmps",
                                          tag="mm")
                            for kk in range(FG):
                                nc.tensor.matmul(
                                    ps[:],
                                    lw_sb[:, kk, g * 128:(g + 1) * 128],
                                    phi[kk][:, nch * PC:(nch + 1) * PC],
                                    start=(kk == 0), stop=(kk == FG - 1))
                            sl = h[g][:, nch * PC:(nch + 1) * PC]
                            nc.vector.tensor_scalar(
                                sl, ps[:],
                                lb_sb[:, li * FG + g:li * FG + g + 1],
                                None, ALU.add)
                    uv_feed(2)
                    srms_relu(h, phi)
                    uv_feed(2)

                # coefs (t-major, lags < L only) = phi.T @ out_w
                ow_sb = mp.tile([128, FG, DH], bf16)
                ob_sb = mp.tile([1, DH], f32)
                dec_sb = mp.tile([128, LC], f32)
                nc.sync.dma_start(ow_sb[:], out_w3[:])
                nc.sync.dma_start(ob_sb[:], outb[:])
                nc.sync.dma_start(dec_sb[:], decay[:])
                obb = mp.tile([128, DH], f32)
                psb = mps.tile([128, DH], f32, name="bc2ps", tag="bc")
                nc.tensor.matmul(psb[:], one_rowf[:], ob_sb[:],
                                 start=True, stop=True)
                nc.vector.tensor_copy(obb[:], psb[:])
                uv_feed(1)
                for m in range(LC):
                    uv_feed(1)
                    ps = mps.tile([128, DH], f32, name="mm2ps", tag="mm")
                    for kk in range(FG):
                        nc.tensor.matmul(
                            ps[:], phi[kk][:, m * 128:(m + 1) * 128],
                            ow_sb[:, kk, :], start=(kk == 0),
                            stop=(kk == FG - 1))
                    ac = mp.tile([128, DH], f32, name="ac", tag="ac")
                    nc.vector.tensor_add(ac[:], ps[:], obb[:])
                    nc.vector.tensor_scalar(
                        acoef[:, m, :], ac[:], dec_sb[:, m:m + 1],
                        None, ALU.mult)
                uv_feed(ROWS // 512)

            # wf head-piece: loads during phase 2 (persist pool, so no
            # dependence on wd's release), letting the inverse phase
            # start before the full wf load completes.
            for j in range(WFA):
                nc.scalar.dma_start(wfa_sb[:, j, :], wf_d[:, j, :])

            # ---------------- forward DFTs + complex multiply -----------
            with (tc.tile_pool(name="fwd", bufs=1) as fp_,
                  tc.tile_pool(name="fwdx", bufs=2) as fpx,
                  tc.tile_pool(name="fwdv", bufs=2) as fpv,
                  tc.tile_pool(name="fwd2", bufs=4) as fp2,
                  tc.tile_pool(name="fwd_ps", bufs=4, space="PSUM") as fps):
                asp = fp_.tile([128, 2 * KC, DH], bf16)    # kernel spectrum
                # A: Ar/Ai m-tiles, contraction over L lags only
                for m in range(2 * KC - 1):
                    ps = fps.tile([128, DH], f32, name="aps", tag="aps")
                    for kc in range(LC):
                        nc.tensor.matmul(
                            ps[:], wd_sb[:, kc, m * 128:(m + 1) * 128],
                            acoef[:, kc, :], start=(kc == 0),
                            stop=(kc == LC - 1))
                    nc.scalar.activation(asp[:, m, :], ps[:], AFT.Copy)
                for b in range(B):
                    vb = fpv.tile([128, N // 128, DH], bf16, name="vb",
                                  tag="vb")
                    nc.sync.dma_start(
                        vb[:], v_d[:, b * (N // 128):(b + 1) * (N // 128), :])
                    xsp = fpx.tile([128, 2 * KC, DH], bf16, name="xsp",
                                   tag="xsp")
                    # D: X = DFT(v_b); interleave re/im pairs for E
                    for j in range(KC):
                        ms = (j,) if j == KC - 1 else (j, KC + j)
                        for m in ms:
                            ps = fps.tile([128, DH], f32, name="dps",
                                          tag="dps")
                            for kc in range(N // 128):
                                nc.tensor.matmul(
                                    ps[:],
                                    wd_sb[:, kc, m * 128:(m + 1) * 128],
                                    vb[:, kc, :], start=(kc == 0),
                                    stop=(kc == N // 128 - 1))
                            nc.scalar.activation(xsp[:, m, :], ps[:],
                                                 AFT.Copy)
                        # E: P = A * X (complex), in place over xsp
                        xr, xi = xsp[:, j, :], xsp[:, KC + j, :]
                        ar, ai = asp[:, j, :], asp[:, KC + j, :]
                        if j == KC - 1:
                            nc.vector.tensor_mul(xr, ar, xr)
                            nc.sync.dma_start(
                                psp_d[:, b * 2 * KC + j, :], xr)
                            continue
                        t1 = fp2.tile([128, DH], bf16, name="t1", tag="t1")
                        t2 = fp2.tile([128, DH], bf16, name="t2", tag="t2")
                        t3 = fp2.tile([128, DH], bf16, name="t3", tag="t3")
                        nc.vector.tensor_mul(t1[:], ar, xr)
                        nc.vector.tensor_mul(t2[:], ai, xi)
                        nc.vector.tensor_mul(t3[:], ar, xi)
                        nc.vector.tensor_mul(xi, ai, xr)
                        nc.vector.tensor_sub(xr, t1[:], t2[:])
                        nc.vector.tensor_add(xi, t3[:], xi)
                        nc.sync.dma_start(psp_d[:, b * 2 * KC + j, :], xr)
                        nc.sync.dma_start(
                            psp_d[:, b * 2 * KC + KC + j, :], xi)
            wd_pool.__exit__(None, None, None)


            # ---------------- inverse DFT + gate + o-projection ---------
            with (tc.tile_pool(name="wf", bufs=1) as wfp,
                  tc.tile_pool(name="inv", bufs=2) as ip_,
                  tc.tile_pool(name="invg", bufs=2) as gp_,
                  tc.tile_pool(name="invw", bufs=1) as owp,
                  tc.tile_pool(name="invs", bufs=4) as isp,
                  tc.tile_pool(name="inv_ps", bufs=4, space="PSUM") as ips):
                NJ = 2 * KC - 1
                wf_sb = wfp.tile([128, NJ - WFA, N], bf16)
                for j in range(NJ - WFA):  # tail: F pipelines with the load
                    nc.scalar.dma_start(wf_sb[:, j, :], wf_d[:, WFA + j, :])

                def wf_j(j):
                    return wfa_sb[:, j, :] if j < WFA else wf_sb[:, j - WFA, :]
                ow_sb = owp.tile([128, DH // 128, E], bf16)
                nc.sync.dma_start(ow_sb[:], o_w3[:])
                for b in range(B):
                    pb = ip_.tile([128, NJ, DH], bf16, name="pb", tag="pb")
                    ub = ip_.tile([128, DH // 128, N], bf16, name="ub",
                                  tag="ub")
                    nc.sync.dma_start(
                        pb[:], psp_d[:, b * 2 * KC:b * 2 * KC + NJ, :])
                    nc.sync.dma_start(
                        ub[:], uT_d[:, :, b * N:(b + 1) * N])
                    gb = gp_.tile([128, DH // 128, N], bf16, name="gb",
                                  tag="gb")
                    # F: tv^T = sum_k P * WF ; gate with u in the evict
                    for m in range(DH // 128):
                        for tch in range(N // 512):
                            ps = ips.tile([128, 512], f32, name="fps",
                                          tag="fps")
                            for j in range(NJ):
                                nc.tensor.matmul(
                                    ps[:], pb[:, j, m * 128:(m + 1) * 128],
                                    wf_j(j)[:, tch * 512:(tch + 1) * 512],
                                    start=(j == 0), stop=(j == NJ - 1))
                            sl = slice(tch * 512, (tch + 1) * 512)
                            nc.vector.tensor_mul(
                                gb[:, m, sl], ps[:], ub[:, m, sl])
                    # H: partial o-projection out_b = g^T @ o_w
                    for mo in range(N // 128):
                        for ech in range(E // 512):
                            ps = ips.tile([128, 512], f32, name="hps",
                                          tag="hps")
                            for kc in range(DH // 128):
                                nc.tensor.matmul(
                                    ps[:], gb[:, kc, mo * 128:(mo + 1) * 128],
                                    ow_sb[:, kc, ech * 512:(ech + 1) * 512],
                                    start=(kc == 0), stop=(kc == DH // 128 - 1))
                            ot = isp.tile([128, 512], f32, name="ot",
                                          tag="ot")
                            nc.scalar.activation(ot[:], ps[:], AFT.Copy)
                            nc.sync.dma_start(
                                out[:, b * (N // 128) + mo,
                                    ech * 512:(ech + 1) * 512], ot[:])

    nc.compile()
    return nc


def _get_nc():
    if "nc" not in _CACHE:
        _CACHE["nc"] = _build()
    return _CACHE["nc"]


def kernel(x, u_w, u_b, v_w, v_b, o_w, o_b,
           pos_w, pos_b, lw0, lb0, lw1, lb1, lw2, lb2, out_w, out_b):
    from concourse.bass_utils import run_bass_kernel_spmd

    wd3, wf3, decay_t = _consts()
    x_flat = np.asarray(x, np.float32).reshape(ROWS, E)
    xTa = np.zeros((KA, ROWS), np.float32)
    xTa[:E] = x_flat.T
    xTa[E] = 1.0
    xTa3 = _t3(xTa, bfl)

    p_aug = np.stack([np.arange(L, dtype=np.float32),
                      np.ones(L, np.float32)])
    pw_aug = np.concatenate([pos_w, pos_b[None, :]], 0).astype(np.float32)
    lbs = np.concatenate(
        [bb.reshape(R // 128, 128).T for bb in (lb0, lb1, lb2)],
        axis=1).astype(np.float32)

    in_maps = []
    for h in range(H):
        sl = slice(h * DH, (h + 1) * DH)
        u_wa = np.zeros((KA, DH), np.float32)
        u_wa[:E] = u_w[:, sl]
        u_wa[E] = u_b[sl]
        v_wa = np.zeros((KA, DH), np.float32)
        v_wa[:E] = v_w[:, sl]
        v_wa[E] = v_b[sl]
        in_maps.append(dict(
            xTa=xTa3, u_wa=_t3(u_wa, bfl), v_wa=_t3(v_wa, bfl),
            u_b3=np.ascontiguousarray(
                u_b[sl].reshape(DH // 128, 128).T).astype(np.float32),
            o_w3=_t3(np.ascontiguousarray(o_w[sl, :]).astype(np.float32), bfl),
            wd=wd3, wf=wf3,
            p_aug=p_aug, pw_aug=pw_aug,
            lw0=_t3(lw0, bfl), lw1=_t3(lw1, bfl), lw2=_t3(lw2, bfl), lbs=lbs,
            out_w3=_t3(np.ascontiguousarray(out_w[:, sl]), bfl),
            outb=np.ascontiguousarray(out_b[None, sl]).astype(np.float32),
            decay=decay_t,
        ))

    nc = _get_nc()
    res = run_bass_kernel_spmd(nc, in_maps, core_ids=list(range(8)),
                               trace=bool(_CACHE.get("trace")))
    _CACHE["last_res"] = res
    acc = np.zeros((ROWS, E), np.float32)
    for i in range(H):
        acc += _from3(res.results[i]["out"])
    acc += o_b[None, :]
    return acc.reshape(B, N, E)


# revision 10
# speedup vs baseline: 28041.6292x; 1.0494x over previous
"""GTU (gated Toeplitz unit) Bass kernel for 8 TRN2 NeuronCores — v2.

Sharding: tensor-parallel over heads (H=8 -> 1 head/core); host sums the
8 partial o-projections.

vs the fp32 baseline (4.98 ms -> 0.94 ms simulated):
- All matmuls in bf16 (1 PE cycle/row vs 4 for fp32); fp32 PSUM
  accumulation; norm math in the RPE MLP stays fp32.
- Kernel lags truncated at L=512 (decay gamma^512 ~ 5.8e-3), shrinking
  the circular conv from 4096 to M2=2560 points and the RPE MLP to the
  512 positions that survive the decay.
- One SBUF-resident DFT matrix per phase, loaded once (not per batch)
  and prefetched on the second (Activation) DMA queue; forward spectra,
  complex multiply and gate all stay on-chip; only the P spectrum
  round-trips DRAM between the two DFT phases. The Nyquist bin lands on a chunk
  boundary, so the all-zero Nyquist-sine chunk is skipped everywhere.
- u/v projections fused into one pass streaming x^T tiles used as both
  moving (uT) and stationary (v) matmul operands, overlapped with the
  serial MLP chains to keep the PE fed.
"""

import numpy as np
import ml_dtypes

B, N, E = 4, 2048, 1024
H = 8
D1 = 3 * E
DH = D1 // H            # 384
R = 512
GAMMA = 0.99
EPS = 1e-8
L = 512                 # truncated kernel lags (4*128)
LC = L // 128           # 4
M2 = 2560               # circular conv length >= N + L - 1
KH = M2 // 2 + 1        # 1281 rfft bins
KC = 11                 # freq chunks of 128 (pad 1281 -> 1408)
KP = KC * 128           # 1408
ROWS = B * N            # 8192
KA = 1152               # augmented contraction for x (bias row), 9*128

_CACHE = {}

bfl = ml_dtypes.bfloat16


def _t3(a, dtype=np.float32):
    """(M, N) -> (128, M/128, N) partition-tiled layout."""
    m, n = a.shape
    assert m % 128 == 0
    return np.ascontiguousarray(
        a.reshape(m // 128, 128, n).transpose(1, 0, 2)).astype(dtype)


def _from3(a):
    p, m, n = a.shape
    return np.ascontiguousarray(
        np.asarray(a, np.float32).transpose(1, 0, 2)).reshape(m * 128, n)


def _consts():
    if "dft" in _CACHE:
        return _CACHE["dft"]
    t = np.arange(N, dtype=np.float64)[:, None]
    k = np.arange(KP, dtype=np.float64)[None, :]
    mask = (k <= (KH - 1)).astype(np.float64)
    ang = 2.0 * np.pi * t * k / M2
    cr = np.cos(ang) * mask
    ci = -np.sin(ang) * mask
    wd = np.concatenate([cr, ci], axis=1)                 # (2048, 3072)

    kk = np.arange(KP, dtype=np.float64)[:, None]
    tt = np.arange(N, dtype=np.float64)[None, :]
    w = np.where((kk == 0) | (kk == M2 // 2), 1.0, 2.0) * (kk <= (KH - 1)) / M2
    ang2 = 2.0 * np.pi * kk * tt / M2
    icos = w * np.cos(ang2)
    isin = (-w * np.sin(ang2))[:KP - 128]  # last sine chunk is all zero
    wf = np.concatenate([icos, isin], axis=0)             # (2688, 2048)

    q1 = np.zeros((128, 128), np.float64)   # out row i <- in row 128-i
    for i in range(1, 128):
        q1[128 - i, i] = 1.0
    q2 = np.zeros((128, 128), np.float64)   # out row 0 <- in row 0
    q2[0, 0] = 1.0

    decay = GAMMA ** np.arange(L, dtype=np.float64)       # lag 0 -> 1.0
    decay_t = decay.reshape(LC, 128).T                    # (128, 6)
    _CACHE["dft"] = (_t3(wd, bfl), _t3(wf, bfl), decay_t.astype(np.float32),
                     q1.astype(bfl), q2.astype(bfl))
    return _CACHE["dft"]


def _build():
    import concourse.bass as bass
    import concourse.mybir as mybir
    import concourse.tile as tile
    from concourse import bacc

    AFT = mybir.ActivationFunctionType
    ALU = mybir.AluOpType
    f32 = mybir.dt.float32
    f32r = mybir.dt.float32r
    bf16 = mybir.dt.bfloat16

    nc = bacc.Bacc(None, target_bir_lowering=False, debug=False, num_devices=8)

    def din(name, shape, dt=f32):
        return nc.dram_tensor(name, list(shape), dt, kind="ExternalInput")

    def dint(name, shape, dt=bf16):
        return nc.dram_tensor(name, list(shape), dt)

    # inputs
    xTa = din("xTa", (128, KA // 128, ROWS), bf16)
    u_wa = din("u_wa", (128, KA // 128, DH), bf16)
    u_b3 = din("u_b3", (128, DH // 128))
    v_wa = din("v_wa", (128, KA // 128, DH), bf16)
    o_w3 = din("o_w3", (128, DH // 128, E), bf16)
    wd_d = din("wd", (128, N // 128, 2 * KP), bf16)
    wf_d = din("wf", (128, 2 * KC - 1, N), bf16)
    p_aug = din("p_aug", (2, L))
    pw_aug = din("pw_aug", (2, R))
    lws = [din(f"lw{i}", (128, R // 128, R), bf16) for i in range(3)]
    lbs = din("lbs", (128, 3 * (R // 128)))
    out_w3 = din("out_w3", (128, R // 128, DH), bf16)
    outb = din("outb", (1, DH))
    decay = din("decay", (128, LC))
    q1_d = din("q1", (128, 128), bf16)
    q2_d = din("q2", (128, 128), bf16)
    out = nc.dram_tensor("out", [128, ROWS // 128, E], f32,
                         kind="ExternalOutput")

    # dram temps (bf16)
    uT_d = dint("uT_d", (128, DH // 128, ROWS))
    v_d = dint("v_d", (128, ROWS // 128, DH))
    psp_d = dint("psp_d", (128, B * 2 * KC, DH))

    FG = R // 128             # 4 feature groups
    PC = 256                  # MLP position-chunk width
    NCH = L // PC             # 2 position chunks in the (truncated) MLP

    with tile.TileContext(nc) as tc, nc.allow_low_precision(
            reason="bf16 pipeline validated against fp32 reference"):
        with tc.tile_pool(name="persist", bufs=1) as pp:
            acoef = pp.tile([128, LC, DH], bf16)   # truncated decayed coefs

            # wd is loaded up front: its pool sits above the phase-0/1
            # working set, so the 12.6MB DMA overlaps the MLP + u/v phase
            # instead of stalling the DFT phase behind it.
            uw_sb = pp.tile([128, KA // 128, DH], bf16)
            vw_sb = pp.tile([128, KA // 128, DH], bf16)
            ub_sb = pp.tile([128, DH // 128], f32)
            nc.sync.dma_start(ub_sb[:], u_b3[:])
            WFA = 2
            wfa_sb = pp.tile([128, WFA, N], bf16)
            q1_sb = pp.tile([128, 128], bf16)
            q2_sb = pp.tile([128, 128], bf16)
            nc.sync.dma_start(q1_sb[:], q1_d[:])
            nc.sync.dma_start(q2_sb[:], q2_d[:])
            nc.scalar.dma_start(uw_sb[:], u_wa[:])
            nc.scalar.dma_start(vw_sb[:], v_wa[:])
            wd_pool = tc.tile_pool(name="wd", bufs=1)
            wdp = wd_pool.__enter__()
            wd_sb = wdp.tile([128, N // 128, 2 * KP], bf16)
            # wd chunks are fed one-per-uv-group through the SP queue: on
            # the Act queue they would sit ahead of the MLP's activations
            # in the Act stream and stall its first norm by ~14us.

            # ------- RPE MLP + u/v projections (concurrent engines) -----
            # The MLP's serial norm->matmul chains leave the PE idle; the
            # u/v GEMMs stream through the same window and fill it.
            with (tc.tile_pool(name="mlp", bufs=1) as mp,
                  tc.tile_pool(name="mlp_ps", bufs=1, space="PSUM") as mps,
                  tc.tile_pool(name="uvx", bufs=3) as xp,
                  tc.tile_pool(name="uvs", bufs=4) as sp,
                  tc.tile_pool(name="uv_ps", bufs=2, space="PSUM") as ups):
                # Engines run their streams in order, so the serial MLP
                # chain must be INTERLEAVED with the u/v GEMM groups at
                # emission time or it just runs after them. uv_feed(k)
                # emits the next k groups; it is called between MLP
                # pipeline stages.
                uv_pending = list(range(ROWS // 512))

                def uv_group(grp):
                    xt = xp.tile([128, KA // 128, 512], bf16, name="xt",
                                 tag="xt")
                    nc.sync.dma_start(
                        xt[:], xTa[:, :, grp * 512:(grp + 1) * 512])
                    nc.sync.dma_start(wd_sb[:, grp, :], wd_d[:, grp, :])
                    # uT tile: out[M=DH, N=512 rows]
                    for m in range(DH // 128):
                        ps = ups.tile([128, 512], f32, name="bps", tag="bps")
                        for kc in range(KA // 128 - 1):  # bias via evict
                            nc.tensor.matmul(
                                ps[:], uw_sb[:, kc, m * 128:(m + 1) * 128],
                                xt[:, kc, :], start=(kc == 0),
                                stop=(kc == KA // 128 - 2))
                        ut = sp.tile([128, 512], bf16, name="ut", tag="ut")
                        nc.scalar.activation(ut[:], ps[:], AFT.Silu,
                                             bias=ub_sb[:, m:m + 1])
                        nc.sync.dma_start(
                            uT_d[:, m, grp * 512:(grp + 1) * 512], ut[:])
                    # v tiles: out[M=128 rows, N=DH]
                    for rs in range(4):
                        ps = ups.tile([128, DH], f32, name="cps", tag="cps")
                        for kc in range(KA // 128):
                            nc.tensor.matmul(
                                ps[:], xt[:, kc, rs * 128:(rs + 1) * 128],
                                vw_sb[:, kc, :], start=(kc == 0),
                                stop=(kc == KA // 128 - 1))
                        vt = sp.tile([128, DH], bf16, name="vt", tag="vt")
                        nc.scalar.activation(vt[:], ps[:], AFT.Silu)
                        nc.sync.dma_start(v_d[:, grp * 4 + rs, :], vt[:])

                def uv_feed(k):
                    for _ in range(min(k, len(uv_pending))):
                        uv_group(uv_pending.pop(0))

                uv_feed(1)
                ones_col = mp.tile([128, 1], bf16)     # K=128 -> M=1 reducer
                nc.vector.memset(ones_col[:], 1.0)
                one_row = mp.tile([1, 128], bf16)      # K=1 -> 128-part bcast
                nc.vector.memset(one_row[:], 1.0)
                one_rowf = mp.tile([1, 128], f32)
                nc.vector.memset(one_rowf[:], 1.0)
                c_sc = mp.tile([1, 1], f32)
                nc.vector.memset(c_sc[:], float(R ** -0.5))
                eps_sc = mp.tile([1, 1], f32)
                nc.vector.memset(eps_sc[:], EPS)

                pa_sb = mp.tile([2, L], f32)
                pw_sb = mp.tile([2, R], f32)
                lb_sb = mp.tile([128, 3 * FG], f32)
                nc.sync.dma_start(pa_sb[:], p_aug[:])
                nc.sync.dma_start(pw_sb[:], pw_aug[:])
                nc.sync.dma_start(lb_sb[:], lbs[:])

                # MLP runs only on the L kept lags; fp32 h, bf16 matmuls
                h = [mp.tile([128, L], f32, name=f"h{g}", tag=f"h{g}")
                     for g in range(FG)]
                # h0 = pos_idx @ pos_w + pos_b (K=2, fp32), feature-major
                for g in range(FG):
                    for nch in range(NCH):
                        ps = mps.tile([128, PC], f32, name="mmps", tag="mm")
                        nc.tensor.matmul(
                            ps[:], pw_sb[:, g * 128:(g + 1) * 128],
                            pa_sb[:, nch * PC:(nch + 1) * PC],
                            start=True, stop=True)
                        nc.vector.tensor_copy(
                            h[g][:, nch * PC:(nch + 1) * PC], ps[:])

                def srms_relu(h_in, phi_out):
                    # s[t] = sum_f h^2 ; factor = 1/(sqrt(s)/sqrt(R) + eps)
                    sq = [mp.tile([128, L], bf16, name=f"sq{g}", tag=f"sq{g}")
                          for g in range(FG)]
                    for g in range(FG):
                        nc.vector.tensor_mul(sq[g][:], h_in[g][:], h_in[g][:])
                    facb = mp.tile([1, L], bf16, name="facb", tag="facb")
                    fac = mp.tile([1, L], f32, name="fac", tag="fac")
                    for nch in range(NCH):
                        ps1 = mps.tile([1, PC], f32, name="redps", tag="red")
                        for g in range(FG):
                            nc.tensor.matmul(
                                ps1[:], ones_col[:],
                                sq[g][:, nch * PC:(nch + 1) * PC],
                                start=(g == 0), stop=(g == FG - 1))
                        sl = fac[:, nch * PC:(nch + 1) * PC]
                        nc.scalar.activation(sl, ps1[:], AFT.Sqrt)
                        nc.vector.tensor_scalar(
                            sl, sl, c_sc[:], eps_sc[:], ALU.mult, ALU.add)
                        nc.vector.reciprocal(
                            facb[:, nch * PC:(nch + 1) * PC], sl)
                    fb = mp.tile([128, L], f32, name="fb", tag="fb")
                    for nch in range(NCH):
                        psb = mps.tile([128, PC], f32, name="bcps", tag="bc")
                        nc.tensor.matmul(
                            psb[:], one_row[:],
                            facb[:, nch * PC:(nch + 1) * PC],
                            start=True, stop=True)
                        nc.vector.tensor_copy(
                            fb[:, nch * PC:(nch + 1) * PC], psb[:])
                    for g in range(FG):
                        nc.vector.tensor_mul(phi_out[g][:], h_in[g][:], fb[:])
                        nc.scalar.activation(
                            phi_out[g][:], phi_out[g][:], AFT.Relu)

                phi = [mp.tile([128, L], bf16, name=f"phi{g}", tag=f"phi{g}")
                       for g in range(FG)]
                uv_feed(1)
                srms_relu(h, phi)
                uv_feed(2)

                lw_sb = mp.tile([128, FG, R], bf16)
                for li in range(3):
                    nc.sync.dma_start(lw_sb[:], lws[li][:])
                    for g in range(FG):
                        for nch in range(NCH):
                            ps = mps.tile([128, PC], f32, name="mmps",
                                          tag="mm")
                            for kk in range(FG):
                                nc.tensor.matmul(
                                    ps[:],
                                    lw_sb[:, kk, g * 128:(g + 1) * 128],
                                    phi[kk][:, nch * PC:(nch + 1) * PC],
                                    start=(kk == 0), stop=(kk == FG - 1))
                            sl = h[g][:, nch * PC:(nch + 1) * PC]
                            nc.vector.tensor_scalar(
                                sl, ps[:],
                                lb_sb[:, li * FG + g:li * FG + g + 1],
                                None, ALU.add)
                    uv_feed(2)
                    srms_relu(h, phi)
                    uv_feed(2)

                # coefs (t-major, lags < L only) = phi.T @ out_w
                ow_sb = mp.tile([128, FG, DH], bf16)
                ob_sb = mp.tile([1, DH], f32)
                dec_sb = mp.tile([128, LC], f32)
                nc.sync.dma_start(ow_sb[:], out_w3[:])
                nc.sync.dma_start(ob_sb[:], outb[:])
                nc.sync.dma_start(dec_sb[:], decay[:])
                obb = mp.tile([128, DH], f32)
                psb = mps.tile([128, DH], f32, name="bc2ps", tag="bc")
                nc.tensor.matmul(psb[:], one_rowf[:], ob_sb[:],
                                 start=True, stop=True)
                nc.vector.tensor_copy(obb[:], psb[:])
                uv_feed(1)
                for m in range(LC):
                    uv_feed(1)
                    ps = mps.tile([128, DH], f32, name="mm2ps", tag="mm")
                    for kk in range(FG):
                        nc.tensor.matmul(
                            ps[:], phi[kk][:, m * 128:(m + 1) * 128],
                            ow_sb[:, kk, :], start=(kk == 0),
                            stop=(kk == FG - 1))
                    ac = mp.tile([128, DH], f32, name="ac", tag="ac")
                    nc.vector.tensor_add(ac[:], ps[:], obb[:])
                    nc.vector.tensor_scalar(
                        acoef[:, m, :], ac[:], dec_sb[:, m:m + 1],
                        None, ALU.mult)
                uv_feed(ROWS // 512)

            # wf head-piece: loads during phase 2 (persist pool, so no
            # dependence on wd's release), letting the inverse phase
            # start before the full wf load completes.
            for j in range(WFA):
                nc.scalar.dma_start(wfa_sb[:, j, :], wf_d[:, j, :])

            # ---------------- forward DFTs + complex multiply -----------
            with (tc.tile_pool(name="fwd", bufs=1) as fp_,
                  tc.tile_pool(name="fwdx", bufs=2) as fpx,
                  tc.tile_pool(name="fwdv", bufs=2) as fpv,
                  tc.tile_pool(name="fwd2", bufs=4) as fp2,
                  tc.tile_pool(name="fwd_ps", bufs=4, space="PSUM") as fps):
                asp = fp_.tile([128, 2 * KC, DH], bf16)    # kernel spectrum
                # A: Ar/Ai m-tiles, contraction over L lags only
                for m in range(2 * KC - 1):
                    ps = fps.tile([128, DH], f32, name="aps", tag="aps")
                    for kc in range(LC):
                        nc.tensor.matmul(
                            ps[:], wd_sb[:, kc, m * 128:(m + 1) * 128],
                            acoef[:, kc, :], start=(kc == 0),
                            stop=(kc == LC - 1))
                    nc.scalar.activation(asp[:, m, :], ps[:], AFT.Copy)
                for b in range(B):
                    vb = fpv.tile([128, N // 128, DH], bf16, name="vb",
                                  tag="vb")
                    nc.sync.dma_start(
                        vb[:], v_d[:, b * (N // 128):(b + 1) * (N // 128), :])
                    # Reflection fold: cos(2pi(M2-t)k/M2)=cos(2pi tk/M2)
                    # and sin flips sign, so t in [512,1280) absorbs its
                    # partner t'=M2-t (in (1280,2048)) as ve=v+rev(v') /
                    # vo=v-rev(v'); rev is a permutation matmul (row i <-
                    # row 128-i of chunk 19-c, row 0 <- row 0 of 20-c).
                    # D then contracts 11 cos / 10 sin chunks, not 16.
                    vf = fp_.tile([128, 13, DH], bf16, name="vf", tag="vf")
                    for ci, c in enumerate(range(4, 10)):
                        ps = fps.tile([128, DH], f32, name="dps", tag="dps")
                        only = (c == 4)   # partner row 0 is t=2048 (zero)
                        nc.tensor.matmul(ps[:], q1_sb[:], vb[:, 19 - c, :],
                                         start=True, stop=only)
                        if not only:
                            nc.tensor.matmul(ps[:], q2_sb[:],
                                             vb[:, 20 - c, :],
                                             start=False, stop=True)
                        nc.vector.tensor_add(vf[:, ci, :], vb[:, c, :],
                                             ps[:])
                        nc.vector.tensor_sub(vf[:, 6 + ci, :], vb[:, c, :],
                                             ps[:])
                    ps10 = fps.tile([128, DH], f32, name="dps", tag="dps")
                    nc.tensor.matmul(ps10[:], q2_sb[:], vb[:, 10, :],
                                     start=True, stop=True)
                    nc.scalar.activation(vf[:, 12, :], ps10[:], AFT.Copy)
                    # (src tensor, data chunk, wd row-chunk)
                    cos_src = ([(vb, kc, kc) for kc in range(4)]
                               + [(vf, ci, 4 + ci) for ci in range(6)]
                               + [(vf, 12, 10)])
                    sin_src = ([(vb, kc, kc) for kc in range(4)]
                               + [(vf, 6 + ci, 4 + ci) for ci in range(6)])
                    xsp = fpx.tile([128, 2 * KC, DH], bf16, name="xsp",
                                   tag="xsp")
                    # D: X = DFT(v_b); interleave re/im pairs for E
                    for j in range(KC):
                        ms = (j,) if j == KC - 1 else (j, KC + j)
                        for m in ms:
                            src = cos_src if m < KC else sin_src
                            ps = fps.tile([128, DH], f32, name="dps",
                                          tag="dps")
                            for ki, (st, dc, wr) in enumerate(src):
                                nc.tensor.matmul(
                                    ps[:],
                                    wd_sb[:, wr, m * 128:(m + 1) * 128],
                                    st[:, dc, :], start=(ki == 0),
                                    stop=(ki == len(src) - 1))
                            nc.scalar.activation(xsp[:, m, :], ps[:],
                                                 AFT.Copy)
                        # E: P = A * X (complex), in place over xsp
                        xr, xi = xsp[:, j, :], xsp[:, KC + j, :]
                        ar, ai = asp[:, j, :], asp[:, KC + j, :]
                        if j == KC - 1:
                            nc.vector.tensor_mul(xr, ar, xr)
                            nc.sync.dma_start(
                                psp_d[:, b * 2 * KC + j, :], xr)
                            continue
                        t1 = fp2.tile([128, DH], bf16, name="t1", tag="t1")
                        t2 = fp2.tile([128, DH], bf16, name="t2", tag="t2")
                        t3 = fp2.tile([128, DH], bf16, name="t3", tag="t3")
                        nc.vector.tensor_mul(t1[:], ar, xr)
                        nc.vector.tensor_mul(t2[:], ai, xi)
                        nc.vector.tensor_mul(t3[:], ar, xi)
                        nc.vector.tensor_mul(xi, ai, xr)
                        nc.vector.tensor_sub(xr, t1[:], t2[:])
                        nc.vector.tensor_add(xi, t3[:], xi)
                        nc.sync.dma_start(psp_d[:, b * 2 * KC + j, :], xr)
                        nc.sync.dma_start(
                            psp_d[:, b * 2 * KC + KC + j, :], xi)
            wd_pool.__exit__(None, None, None)


            # ---------------- inverse DFT + gate + o-projection ---------
            with (tc.tile_pool(name="wf", bufs=1) as wfp,
                  tc.tile_pool(name="inv", bufs=2) as ip_,
                  tc.tile_pool(name="invg", bufs=2) as gp_,
                  tc.tile_pool(name="invw", bufs=1) as owp,
                  tc.tile_pool(name="invs", bufs=4) as isp,
                  tc.tile_pool(name="inv_ps", bufs=4, space="PSUM") as ips):
                NJ = 2 * KC - 1
                wf_sb = wfp.tile([128, NJ - WFA, N], bf16)
                for j in range(NJ - WFA):  # tail: F pipelines with the load
                    nc.scalar.dma_start(wf_sb[:, j, :], wf_d[:, WFA + j, :])

                def wf_j(j):
                    return wfa_sb[:, j, :] if j < WFA else wf_sb[:, j - WFA, :]
                ow_sb = owp.tile([128, DH // 128, E], bf16)
                nc.sync.dma_start(ow_sb[:], o_w3[:])
                for b in range(B):
                    pb = ip_.tile([128, NJ, DH], bf16, name="pb", tag="pb")
                    ub = ip_.tile([128, DH // 128, N], bf16, name="ub",
                                  tag="ub")
                    nc.sync.dma_start(
                        pb[:], psp_d[:, b * 2 * KC:b * 2 * KC + NJ, :])
                    nc.sync.dma_start(
                        ub[:], uT_d[:, :, b * N:(b + 1) * N])
                    gb = gp_.tile([128, DH // 128, N], bf16, name="gb",
                                  tag="gb")
                    # F: tv^T = sum_k P * WF ; gate with u in the evict
                    for m in range(DH // 128):
                        for tch in range(N // 512):
                            ps = ips.tile([128, 512], f32, name="fps",
                                          tag="fps")
                            for j in range(NJ):
                                nc.tensor.matmul(
                                    ps[:], pb[:, j, m * 128:(m + 1) * 128],
                                    wf_j(j)[:, tch * 512:(tch + 1) * 512],
                                    start=(j == 0), stop=(j == NJ - 1))
                            sl = slice(tch * 512, (tch + 1) * 512)
                            nc.vector.tensor_mul(
                                gb[:, m, sl], ps[:], ub[:, m, sl])
                    # H: partial o-projection out_b = g^T @ o_w
                    for mo in range(N // 128):
                        for ech in range(E // 512):
                            ps = ips.tile([128, 512], f32, name="hps",
                                          tag="hps")
                            for kc in range(DH // 128):
                                nc.tensor.matmul(
                                    ps[:], gb[:, kc, mo * 128:(mo + 1) * 128],
                                    ow_sb[:, kc, ech * 512:(ech + 1) * 512],
                                    start=(kc == 0), stop=(kc == DH // 128 - 1))
                            ot = isp.tile([128, 512], f32, name="ot",
                                          tag="ot")
                            nc.scalar.activation(ot[:], ps[:], AFT.Copy)
                            nc.sync.dma_start(
                                out[:, b * (N // 128) + mo,
                                    ech * 512:(ech + 1) * 512], ot[:])

    nc.compile()
    return nc


def _get_nc():
    if "nc" not in _CACHE:
        _CACHE["nc"] = _build()
    return _CACHE["nc"]


def kernel(x, u_w, u_b, v_w, v_b, o_w, o_b,
           pos_w, pos_b, lw0, lb0, lw1, lb1, lw2, lb2, out_w, out_b):
    from concourse.bass_utils import run_bass_kernel_spmd

    wd3, wf3, decay_t, q1h, q2h = _consts()
    x_flat = np.asarray(x, np.float32).reshape(ROWS, E)
    xTa = np.zeros((KA, ROWS), np.float32)
    xTa[:E] = x_flat.T
    xTa[E] = 1.0
    xTa3 = _t3(xTa, bfl)

    p_aug = np.stack([np.arange(L, dtype=np.float32),
                      np.ones(L, np.float32)])
    pw_aug = np.concatenate([pos_w, pos_b[None, :]], 0).astype(np.float32)
    lbs = np.concatenate(
        [bb.reshape(R // 128, 128).T for bb in (lb0, lb1, lb2)],
        axis=1).astype(np.float32)

    in_maps = []
    for h in range(H):
        sl = slice(h * DH, (h + 1) * DH)
        u_wa = np.zeros((KA, DH), np.float32)
        u_wa[:E] = u_w[:, sl]
        u_wa[E] = u_b[sl]
        v_wa = np.zeros((KA, DH), np.float32)
        v_wa[:E] = v_w[:, sl]
        v_wa[E] = v_b[sl]
        in_maps.append(dict(
            xTa=xTa3, u_wa=_t3(u_wa, bfl), v_wa=_t3(v_wa, bfl),
            u_b3=np.ascontiguousarray(
                u_b[sl].reshape(DH // 128, 128).T).astype(np.float32),
            o_w3=_t3(np.ascontiguousarray(o_w[sl, :]).astype(np.float32), bfl),
            wd=wd3, wf=wf3,
            p_aug=p_aug, pw_aug=pw_aug,
            lw0=_t3(lw0, bfl), lw1=_t3(lw1, bfl), lw2=_t3(lw2, bfl), lbs=lbs,
            out_w3=_t3(np.ascontiguousarray(out_w[:, sl]), bfl),
            outb=np.ascontiguousarray(out_b[None, sl]).astype(np.float32),
            decay=decay_t, q1=q1h, q2=q2h,
        ))

    nc = _get_nc()
    res = run_bass_kernel_spmd(nc, in_maps, core_ids=list(range(8)),
                               trace=bool(_CACHE.get("trace")))
    _CACHE["last_res"] = res
    acc = np.zeros((ROWS, E), np.float32)
    for i in range(H):
        acc += _from3(res.results[i]["out"])
    acc += o_b[None, :]
    return acc.reshape(B, N, E)


# revision 11
# speedup vs baseline: 28054.2023x; 1.0004x over previous
"""GTU (gated Toeplitz unit) Bass kernel for 8 TRN2 NeuronCores — v2.

Sharding: tensor-parallel over heads (H=8 -> 1 head/core); host sums the
8 partial o-projections.

vs the fp32 baseline (4.98 ms -> 0.94 ms simulated):
- All matmuls in bf16 (1 PE cycle/row vs 4 for fp32); fp32 PSUM
  accumulation; norm math in the RPE MLP stays fp32.
- Kernel lags truncated at L=512 (decay gamma^512 ~ 5.8e-3), shrinking
  the circular conv from 4096 to M2=2560 points and the RPE MLP to the
  512 positions that survive the decay.
- One SBUF-resident DFT matrix per phase, loaded once (not per batch)
  and prefetched on the second (Activation) DMA queue; forward spectra,
  complex multiply and gate all stay on-chip; only the P spectrum
  round-trips DRAM between the two DFT phases. The Nyquist bin lands on a chunk
  boundary, so the all-zero Nyquist-sine chunk is skipped everywhere.
- u/v projections fused into one pass streaming x^T tiles used as both
  moving (uT) and stationary (v) matmul operands, overlapped with the
  serial MLP chains to keep the PE fed.
"""

import numpy as np
import ml_dtypes

B, N, E = 4, 2048, 1024
H = 8
D1 = 3 * E
DH = D1 // H            # 384
R = 512
GAMMA = 0.99
EPS = 1e-8
L = 512                 # truncated kernel lags (4*128)
LC = L // 128           # 4
M2 = 2560               # circular conv length >= N + L - 1
KH = M2 // 2 + 1        # 1281 rfft bins
KC = 11                 # freq chunks of 128 (pad 1281 -> 1408)
KP = KC * 128           # 1408
ROWS = B * N            # 8192
KA = 1152               # augmented contraction for x (bias row), 9*128

_CACHE = {}

bfl = ml_dtypes.bfloat16


def _t3(a, dtype=np.float32):
    """(M, N) -> (128, M/128, N) partition-tiled layout."""
    m, n = a.shape
    assert m % 128 == 0
    return np.ascontiguousarray(
        a.reshape(m // 128, 128, n).transpose(1, 0, 2)).astype(dtype)


def _from3(a):
    p, m, n = a.shape
    return np.ascontiguousarray(
        np.asarray(a, np.float32).transpose(1, 0, 2)).reshape(m * 128, n)


def _consts():
    if "dft" in _CACHE:
        return _CACHE["dft"]
    t = np.arange(N, dtype=np.float64)[:, None]
    k = np.arange(KP, dtype=np.float64)[None, :]
    mask = (k <= (KH - 1)).astype(np.float64)
    ang = 2.0 * np.pi * t * k / M2
    cr = np.cos(ang) * mask
    ci = -np.sin(ang) * mask
    wd = np.concatenate([cr, ci], axis=1)                 # (2048, 3072)

    kk = np.arange(KP, dtype=np.float64)[:, None]
    tt = np.arange(N, dtype=np.float64)[None, :]
    w = np.where((kk == 0) | (kk == M2 // 2), 1.0, 2.0) * (kk <= (KH - 1)) / M2
    ang2 = 2.0 * np.pi * kk * tt / M2
    icos = w * np.cos(ang2)
    isin = (-w * np.sin(ang2))[:KP - 128]  # last sine chunk is all zero
    wf = np.concatenate([icos, isin], axis=0)             # (2688, 2048)

    q1 = np.zeros((128, 128), np.float64)   # out row i <- in row 128-i
    for i in range(1, 128):
        q1[128 - i, i] = 1.0
    q2 = np.zeros((128, 128), np.float64)   # out row 0 <- in row 0
    q2[0, 0] = 1.0

    decay = GAMMA ** np.arange(L, dtype=np.float64)       # lag 0 -> 1.0
    decay_t = decay.reshape(LC, 128).T                    # (128, 6)
    _CACHE["dft"] = (_t3(wd, bfl), _t3(wf, bfl), decay_t.astype(np.float32),
                     q1.astype(bfl), q2.astype(bfl))
    return _CACHE["dft"]


def _build():
    import concourse.bass as bass
    import concourse.mybir as mybir
    import concourse.tile as tile
    from concourse import bacc

    AFT = mybir.ActivationFunctionType
    ALU = mybir.AluOpType
    f32 = mybir.dt.float32
    f32r = mybir.dt.float32r
    bf16 = mybir.dt.bfloat16

    nc = bacc.Bacc(None, target_bir_lowering=False, debug=False, num_devices=8)

    def din(name, shape, dt=f32):
        return nc.dram_tensor(name, list(shape), dt, kind="ExternalInput")

    def dint(name, shape, dt=bf16):
        return nc.dram_tensor(name, list(shape), dt)

    # inputs
    xTa = din("xTa", (128, KA // 128, ROWS), bf16)
    u_wa = din("u_wa", (128, KA // 128, DH), bf16)
    u_b3 = din("u_b3", (128, DH // 128))
    v_wa = din("v_wa", (128, KA // 128, DH), bf16)
    o_w3 = din("o_w3", (128, DH // 128, E), bf16)
    wd_d = din("wd", (128, N // 128, 2 * KP), bf16)
    wf_d = din("wf", (128, 2 * KC - 1, N), bf16)
    p_aug = din("p_aug", (2, L))
    pw_aug = din("pw_aug", (2, R))
    lws = [din(f"lw{i}", (128, R // 128, R), bf16) for i in range(3)]
    lbs = din("lbs", (128, 3 * (R // 128)))
    out_w3 = din("out_w3", (128, R // 128, DH), bf16)
    outb = din("outb", (1, DH))
    decay = din("decay", (128, LC))
    q1_d = din("q1", (128, 128), bf16)
    q2_d = din("q2", (128, 128), bf16)
    out = nc.dram_tensor("out", [128, ROWS // 128, E], f32,
                         kind="ExternalOutput")

    # dram temps (bf16)
    uT_d = dint("uT_d", (128, DH // 128, ROWS))
    v_d = dint("v_d", (128, ROWS // 128, DH))
    psp_d = dint("psp_d", (128, B * 2 * KC, DH))

    FG = R // 128             # 4 feature groups
    PC = 256                  # MLP position-chunk width
    NCH = L // PC             # 2 position chunks in the (truncated) MLP

    with tile.TileContext(nc) as tc, nc.allow_low_precision(
            reason="bf16 pipeline validated against fp32 reference"):
        with tc.tile_pool(name="persist", bufs=1) as pp:
            acoef = pp.tile([128, LC, DH], bf16)   # truncated decayed coefs

            # wd is loaded up front: its pool sits above the phase-0/1
            # working set, so the 12.6MB DMA overlaps the MLP + u/v phase
            # instead of stalling the DFT phase behind it.
            uw_sb = pp.tile([128, KA // 128, DH], bf16)
            vw_sb = pp.tile([128, KA // 128, DH], bf16)
            ub_sb = pp.tile([128, DH // 128], f32)
            nc.sync.dma_start(ub_sb[:], u_b3[:])
            WFA = 8
            wfa_sb = pp.tile([128, WFA, N], bf16)
            q1_sb = pp.tile([128, 128], bf16)
            q2_sb = pp.tile([128, 128], bf16)
            nc.sync.dma_start(q1_sb[:], q1_d[:])
            nc.sync.dma_start(q2_sb[:], q2_d[:])
            nc.scalar.dma_start(uw_sb[:], u_wa[:])
            nc.scalar.dma_start(vw_sb[:], v_wa[:])
            wd_pool = tc.tile_pool(name="wd", bufs=1)
            wdp = wd_pool.__enter__()
            WDC = 11   # the fold leaves wd row-chunks 11..15 unread
            wd_sb = wdp.tile([128, WDC, 2 * KP], bf16)
            # wd chunks are fed one-per-uv-group through the SP queue: on
            # the Act queue they would sit ahead of the MLP's activations
            # in the Act stream and stall its first norm by ~14us.

            # ------- RPE MLP + u/v projections (concurrent engines) -----
            # The MLP's serial norm->matmul chains leave the PE idle; the
            # u/v GEMMs stream through the same window and fill it.
            with (tc.tile_pool(name="mlp", bufs=1) as mp,
                  tc.tile_pool(name="mlp_ps", bufs=1, space="PSUM") as mps,
                  tc.tile_pool(name="uvx", bufs=3) as xp,
                  tc.tile_pool(name="uvs", bufs=4) as sp,
                  tc.tile_pool(name="uv_ps", bufs=2, space="PSUM") as ups):
                # Engines run their streams in order, so the serial MLP
                # chain must be INTERLEAVED with the u/v GEMM groups at
                # emission time or it just runs after them. uv_feed(k)
                # emits the next k groups; it is called between MLP
                # pipeline stages.
                uv_pending = list(range(ROWS // 512))

                def uv_group(grp):
                    xt = xp.tile([128, KA // 128, 512], bf16, name="xt",
                                 tag="xt")
                    nc.sync.dma_start(
                        xt[:], xTa[:, :, grp * 512:(grp + 1) * 512])
                    if grp < WDC:
                        nc.sync.dma_start(wd_sb[:, grp, :],
                                          wd_d[:, grp, :])
                    # uT tile: out[M=DH, N=512 rows]
                    for m in range(DH // 128):
                        ps = ups.tile([128, 512], f32, name="bps", tag="bps")
                        for kc in range(KA // 128 - 1):  # bias via evict
                            nc.tensor.matmul(
                                ps[:], uw_sb[:, kc, m * 128:(m + 1) * 128],
                                xt[:, kc, :], start=(kc == 0),
                                stop=(kc == KA // 128 - 2))
                        ut = sp.tile([128, 512], bf16, name="ut", tag="ut")
                        nc.scalar.activation(ut[:], ps[:], AFT.Silu,
                                             bias=ub_sb[:, m:m + 1])
                        nc.sync.dma_start(
                            uT_d[:, m, grp * 512:(grp + 1) * 512], ut[:])
                    # v tiles: out[M=128 rows, N=DH]
                    for rs in range(4):
                        ps = ups.tile([128, DH], f32, name="cps", tag="cps")
                        for kc in range(KA // 128):
                            nc.tensor.matmul(
                                ps[:], xt[:, kc, rs * 128:(rs + 1) * 128],
                                vw_sb[:, kc, :], start=(kc == 0),
                                stop=(kc == KA // 128 - 1))
                        vt = sp.tile([128, DH], bf16, name="vt", tag="vt")
                        nc.scalar.activation(vt[:], ps[:], AFT.Silu)
                        nc.sync.dma_start(v_d[:, grp * 4 + rs, :], vt[:])

                def uv_feed(k):
                    for _ in range(min(k, len(uv_pending))):
                        uv_group(uv_pending.pop(0))

                uv_feed(1)
                ones_col = mp.tile([128, 1], bf16)     # K=128 -> M=1 reducer
                nc.vector.memset(ones_col[:], 1.0)
                one_row = mp.tile([1, 128], bf16)      # K=1 -> 128-part bcast
                nc.vector.memset(one_row[:], 1.0)
                one_rowf = mp.tile([1, 128], f32)
                nc.vector.memset(one_rowf[:], 1.0)
                c_sc = mp.tile([1, 1], f32)
                nc.vector.memset(c_sc[:], float(R ** -0.5))
                eps_sc = mp.tile([1, 1], f32)
                nc.vector.memset(eps_sc[:], EPS)

                pa_sb = mp.tile([2, L], f32)
                pw_sb = mp.tile([2, R], f32)
                lb_sb = mp.tile([128, 3 * FG], f32)
                nc.sync.dma_start(pa_sb[:], p_aug[:])
                nc.sync.dma_start(pw_sb[:], pw_aug[:])
                nc.sync.dma_start(lb_sb[:], lbs[:])

                # MLP runs only on the L kept lags; fp32 h, bf16 matmuls
                h = [mp.tile([128, L], f32, name=f"h{g}", tag=f"h{g}")
                     for g in range(FG)]
                # h0 = pos_idx @ pos_w + pos_b (K=2, fp32), feature-major
                for g in range(FG):
                    for nch in range(NCH):
                        ps = mps.tile([128, PC], f32, name="mmps", tag="mm")
                        nc.tensor.matmul(
                            ps[:], pw_sb[:, g * 128:(g + 1) * 128],
                            pa_sb[:, nch * PC:(nch + 1) * PC],
                            start=True, stop=True)
                        nc.vector.tensor_copy(
                            h[g][:, nch * PC:(nch + 1) * PC], ps[:])

                def srms_relu(h_in, phi_out):
                    # s[t] = sum_f h^2 ; factor = 1/(sqrt(s)/sqrt(R) + eps)
                    sq = [mp.tile([128, L], bf16, name=f"sq{g}", tag=f"sq{g}")
                          for g in range(FG)]
                    for g in range(FG):
                        nc.vector.tensor_mul(sq[g][:], h_in[g][:], h_in[g][:])
                    facb = mp.tile([1, L], bf16, name="facb", tag="facb")
                    fac = mp.tile([1, L], f32, name="fac", tag="fac")
                    for nch in range(NCH):
                        ps1 = mps.tile([1, PC], f32, name="redps", tag="red")
                        for g in range(FG):
                            nc.tensor.matmul(
                                ps1[:], ones_col[:],
                                sq[g][:, nch * PC:(nch + 1) * PC],
                                start=(g == 0), stop=(g == FG - 1))
                        sl = fac[:, nch * PC:(nch + 1) * PC]
                        nc.scalar.activation(sl, ps1[:], AFT.Sqrt)
                        nc.vector.tensor_scalar(
                            sl, sl, c_sc[:], eps_sc[:], ALU.mult, ALU.add)
                        nc.vector.reciprocal(
                            facb[:, nch * PC:(nch + 1) * PC], sl)
                    fb = mp.tile([128, L], f32, name="fb", tag="fb")
                    for nch in range(NCH):
                        psb = mps.tile([128, PC], f32, name="bcps", tag="bc")
                        nc.tensor.matmul(
                            psb[:], one_row[:],
                            facb[:, nch * PC:(nch + 1) * PC],
                            start=True, stop=True)
                        nc.vector.tensor_copy(
                            fb[:, nch * PC:(nch + 1) * PC], psb[:])
                    for g in range(FG):
                        nc.vector.tensor_mul(phi_out[g][:], h_in[g][:], fb[:])
                        nc.scalar.activation(
                            phi_out[g][:], phi_out[g][:], AFT.Relu)

                phi = [mp.tile([128, L], bf16, name=f"phi{g}", tag=f"phi{g}")
                       for g in range(FG)]
                uv_feed(1)
                srms_relu(h, phi)
                uv_feed(2)

                lw_sb = mp.tile([128, FG, R], bf16)
                for li in range(3):
                    nc.sync.dma_start(lw_sb[:], lws[li][:])
                    for g in range(FG):
                        for nch in range(NCH):
                            ps = mps.tile([128, PC], f32, name="mmps",
                                          tag="mm")
                            for kk in range(FG):
                                nc.tensor.matmul(
                                    ps[:],
                                    lw_sb[:, kk, g * 128:(g + 1) * 128],
                                    phi[kk][:, nch * PC:(nch + 1) * PC],
                                    start=(kk == 0), stop=(kk == FG - 1))
                            sl = h[g][:, nch * PC:(nch + 1) * PC]
                            nc.vector.tensor_scalar(
                                sl, ps[:],
                                lb_sb[:, li * FG + g:li * FG + g + 1],
                                None, ALU.add)
                    uv_feed(2)
                    srms_relu(h, phi)
                    uv_feed(2)

                # coefs (t-major, lags < L only) = phi.T @ out_w
                ow_sb = mp.tile([128, FG, DH], bf16)
                ob_sb = mp.tile([1, DH], f32)
                dec_sb = mp.tile([128, LC], f32)
                nc.sync.dma_start(ow_sb[:], out_w3[:])
                nc.sync.dma_start(ob_sb[:], outb[:])
                nc.sync.dma_start(dec_sb[:], decay[:])
                obb = mp.tile([128, DH], f32)
                psb = mps.tile([128, DH], f32, name="bc2ps", tag="bc")
                nc.tensor.matmul(psb[:], one_rowf[:], ob_sb[:],
                                 start=True, stop=True)
                nc.vector.tensor_copy(obb[:], psb[:])
                uv_feed(1)
                for m in range(LC):
                    uv_feed(1)
                    ps = mps.tile([128, DH], f32, name="mm2ps", tag="mm")
                    for kk in range(FG):
                        nc.tensor.matmul(
                            ps[:], phi[kk][:, m * 128:(m + 1) * 128],
                            ow_sb[:, kk, :], start=(kk == 0),
                            stop=(kk == FG - 1))
                    ac = mp.tile([128, DH], f32, name="ac", tag="ac")
                    nc.vector.tensor_add(ac[:], ps[:], obb[:])
                    nc.vector.tensor_scalar(
                        acoef[:, m, :], ac[:], dec_sb[:, m:m + 1],
                        None, ALU.mult)
                uv_feed(ROWS // 512)

            # wf head-piece: loads during phase 2 (persist pool, so no
            # dependence on wd's release), letting the inverse phase
            # start before the full wf load completes.
            for j in range(WFA):
                nc.scalar.dma_start(wfa_sb[:, j, :], wf_d[:, j, :])

            # ---------------- forward DFTs + complex multiply -----------
            with (tc.tile_pool(name="fwd", bufs=1) as fp_,
                  tc.tile_pool(name="fwdx", bufs=2) as fpx,
                  tc.tile_pool(name="fwdv", bufs=2) as fpv,
                  tc.tile_pool(name="fwd2", bufs=4) as fp2,
                  tc.tile_pool(name="fwd_ps", bufs=4, space="PSUM") as fps):
                asp = fp_.tile([128, 2 * KC, DH], bf16)    # kernel spectrum
                # A: Ar/Ai m-tiles, contraction over L lags only
                for m in range(2 * KC - 1):
                    ps = fps.tile([128, DH], f32, name="aps", tag="aps")
                    for kc in range(LC):
                        nc.tensor.matmul(
                            ps[:], wd_sb[:, kc, m * 128:(m + 1) * 128],
                            acoef[:, kc, :], start=(kc == 0),
                            stop=(kc == LC - 1))
                    nc.scalar.activation(asp[:, m, :], ps[:], AFT.Copy)
                for b in range(B):
                    vb = fpv.tile([128, N // 128, DH], bf16, name="vb",
                                  tag="vb")
                    nc.sync.dma_start(
                        vb[:], v_d[:, b * (N // 128):(b + 1) * (N // 128), :])
                    # Reflection fold: cos(2pi(M2-t)k/M2)=cos(2pi tk/M2)
                    # and sin flips sign, so t in [512,1280) absorbs its
                    # partner t'=M2-t (in (1280,2048)) as ve=v+rev(v') /
                    # vo=v-rev(v'); rev is a permutation matmul (row i <-
                    # row 128-i of chunk 19-c, row 0 <- row 0 of 20-c).
                    # D then contracts 11 cos / 10 sin chunks, not 16.
                    vf = fp_.tile([128, 13, DH], bf16, name="vf", tag="vf")
                    for ci, c in enumerate(range(4, 10)):
                        ps = fps.tile([128, DH], f32, name="dps", tag="dps")
                        only = (c == 4)   # partner row 0 is t=2048 (zero)
                        nc.tensor.matmul(ps[:], q1_sb[:], vb[:, 19 - c, :],
                                         start=True, stop=only)
                        if not only:
                            nc.tensor.matmul(ps[:], q2_sb[:],
                                             vb[:, 20 - c, :],
                                             start=False, stop=True)
                        nc.vector.tensor_add(vf[:, ci, :], vb[:, c, :],
                                             ps[:])
                        nc.vector.tensor_sub(vf[:, 6 + ci, :], vb[:, c, :],
                                             ps[:])
                    ps10 = fps.tile([128, DH], f32, name="dps", tag="dps")
                    nc.tensor.matmul(ps10[:], q2_sb[:], vb[:, 10, :],
                                     start=True, stop=True)
                    nc.scalar.activation(vf[:, 12, :], ps10[:], AFT.Copy)
                    # (src tensor, data chunk, wd row-chunk)
                    cos_src = ([(vb, kc, kc) for kc in range(4)]
                               + [(vf, ci, 4 + ci) for ci in range(6)]
                               + [(vf, 12, 10)])
                    sin_src = ([(vb, kc, kc) for kc in range(4)]
                               + [(vf, 6 + ci, 4 + ci) for ci in range(6)])
                    xsp = fpx.tile([128, 2 * KC, DH], bf16, name="xsp",
                                   tag="xsp")
                    # D: X = DFT(v_b); interleave re/im pairs for E
                    for j in range(KC):
                        ms = (j,) if j == KC - 1 else (j, KC + j)
                        for m in ms:
                            src = cos_src if m < KC else sin_src
                            ps = fps.tile([128, DH], f32, name="dps",
                                          tag="dps")
                            for ki, (st, dc, wr) in enumerate(src):
                                nc.tensor.matmul(
                                    ps[:],
                                    wd_sb[:, wr, m * 128:(m + 1) * 128],
                                    st[:, dc, :], start=(ki == 0),
                                    stop=(ki == len(src) - 1))
                            nc.scalar.activation(xsp[:, m, :], ps[:],
                                                 AFT.Copy)
                        # E: P = A * X (complex), in place over xsp
                        xr, xi = xsp[:, j, :], xsp[:, KC + j, :]
                        ar, ai = asp[:, j, :], asp[:, KC + j, :]
                        if j == KC - 1:
                            nc.vector.tensor_mul(xr, ar, xr)
                            nc.sync.dma_start(
                                psp_d[:, b * 2 * KC + j, :], xr)
                            continue
                        t1 = fp2.tile([128, DH], bf16, name="t1", tag="t1")
                        t2 = fp2.tile([128, DH], bf16, name="t2", tag="t2")
                        t3 = fp2.tile([128, DH], bf16, name="t3", tag="t3")
                        nc.vector.tensor_mul(t1[:], ar, xr)
                        nc.vector.tensor_mul(t2[:], ai, xi)
                        nc.vector.tensor_mul(t3[:], ar, xi)
                        nc.vector.tensor_mul(xi, ai, xr)
                        nc.vector.tensor_sub(xr, t1[:], t2[:])
                        nc.vector.tensor_add(xi, t3[:], xi)
                        nc.sync.dma_start(psp_d[:, b * 2 * KC + j, :], xr)
                        nc.sync.dma_start(
                            psp_d[:, b * 2 * KC + KC + j, :], xi)
            wd_pool.__exit__(None, None, None)


            # ---------------- inverse DFT + gate + o-projection ---------
            with (tc.tile_pool(name="wf", bufs=1) as wfp,
                  tc.tile_pool(name="inv", bufs=2) as ip_,
                  tc.tile_pool(name="invg", bufs=2) as gp_,
                  tc.tile_pool(name="invw", bufs=1) as owp,
                  tc.tile_pool(name="invs", bufs=4) as isp,
                  tc.tile_pool(name="inv_ps", bufs=4, space="PSUM") as ips):
                NJ = 2 * KC - 1
                wf_sb = wfp.tile([128, NJ - WFA, N], bf16)
                for j in range(NJ - WFA):  # tail: F pipelines with the load
                    nc.scalar.dma_start(wf_sb[:, j, :], wf_d[:, WFA + j, :])

                def wf_j(j):
                    return wfa_sb[:, j, :] if j < WFA else wf_sb[:, j - WFA, :]
                ow_sb = owp.tile([128, DH // 128, E], bf16)
                nc.sync.dma_start(ow_sb[:], o_w3[:])
                for b in range(B):
                    pb = ip_.tile([128, NJ, DH], bf16, name="pb", tag="pb")
                    ub = ip_.tile([128, DH // 128, N], bf16, name="ub",
                                  tag="ub")
                    nc.sync.dma_start(
                        pb[:], psp_d[:, b * 2 * KC:b * 2 * KC + NJ, :])
                    nc.sync.dma_start(
                        ub[:], uT_d[:, :, b * N:(b + 1) * N])
                    gb = gp_.tile([128, DH // 128, N], bf16, name="gb",
                                  tag="gb")
                    # F: tv^T = sum_k P * WF ; gate with u in the evict
                    for m in range(DH // 128):
                        for tch in range(N // 512):
                            ps = ips.tile([128, 512], f32, name="fps",
                                          tag="fps")
                            for j in range(NJ):
                                nc.tensor.matmul(
                                    ps[:], pb[:, j, m * 128:(m + 1) * 128],
                                    wf_j(j)[:, tch * 512:(tch + 1) * 512],
                                    start=(j == 0), stop=(j == NJ - 1))
                            sl = slice(tch * 512, (tch + 1) * 512)
                            nc.vector.tensor_mul(
                                gb[:, m, sl], ps[:], ub[:, m, sl])
                    # H: partial o-projection out_b = g^T @ o_w
                    for mo in range(N // 128):
                        for ech in range(E // 512):
                            ps = ips.tile([128, 512], f32, name="hps",
                                          tag="hps")
                            for kc in range(DH // 128):
                                nc.tensor.matmul(
                                    ps[:], gb[:, kc, mo * 128:(mo + 1) * 128],
                                    ow_sb[:, kc, ech * 512:(ech + 1) * 512],
                                    start=(kc == 0), stop=(kc == DH // 128 - 1))
                            ot = isp.tile([128, 512], f32, name="ot",
                                          tag="ot")
                            nc.scalar.activation(ot[:], ps[:], AFT.Copy)
                            nc.sync.dma_start(
                                out[:, b * (N // 128) + mo,
                                    ech * 512:(ech + 1) * 512], ot[:])

    nc.compile()
    return nc


def _get_nc():
    if "nc" not in _CACHE:
        _CACHE["nc"] = _build()
    return _CACHE["nc"]


def kernel(x, u_w, u_b, v_w, v_b, o_w, o_b,
           pos_w, pos_b, lw0, lb0, lw1, lb1, lw2, lb2, out_w, out_b):
    from concourse.bass_utils import run_bass_kernel_spmd

    wd3, wf3, decay_t, q1h, q2h = _consts()
    x_flat = np.asarray(x, np.float32).reshape(ROWS, E)
    xTa = np.zeros((KA, ROWS), np.float32)
    xTa[:E] = x_flat.T
    xTa[E] = 1.0
    xTa3 = _t3(xTa, bfl)

    p_aug = np.stack([np.arange(L, dtype=np.float32),
                      np.ones(L, np.float32)])
    pw_aug = np.concatenate([pos_w, pos_b[None, :]], 0).astype(np.float32)
    lbs = np.concatenate(
        [bb.reshape(R // 128, 128).T for bb in (lb0, lb1, lb2)],
        axis=1).astype(np.float32)

    in_maps = []
    for h in range(H):
        sl = slice(h * DH, (h + 1) * DH)
        u_wa = np.zeros((KA, DH), np.float32)
        u_wa[:E] = u_w[:, sl]
        u_wa[E] = u_b[sl]
        v_wa = np.zeros((KA, DH), np.float32)
        v_wa[:E] = v_w[:, sl]
        v_wa[E] = v_b[sl]
        in_maps.append(dict(
            xTa=xTa3, u_wa=_t3(u_wa, bfl), v_wa=_t3(v_wa, bfl),
            u_b3=np.ascontiguousarray(
                u_b[sl].reshape(DH // 128, 128).T).astype(np.float32),
            o_w3=_t3(np.ascontiguousarray(o_w[sl, :]).astype(np.float32), bfl),
            wd=wd3, wf=wf3,
            p_aug=p_aug, pw_aug=pw_aug,
            lw0=_t3(lw0, bfl), lw1=_t3(lw1, bfl), lw2=_t3(lw2, bfl), lbs=lbs,
            out_w3=_t3(np.ascontiguousarray(out_w[:, sl]), bfl),
            outb=np.ascontiguousarray(out_b[None, sl]).astype(np.float32),
            decay=decay_t, q1=q1h, q2=q2h,
        ))

    nc = _get_nc()
    res = run_bass_kernel_spmd(nc, in_maps, core_ids=list(range(8)),
                               trace=bool(_CACHE.get("trace")))
    _CACHE["last_res"] = res
    acc = np.zeros((ROWS, E), np.float32)
    for i in range(H):
        acc += _from3(res.results[i]["out"])
    acc += o_b[None, :]
    return acc.reshape(B, N, E)
